# revision 11
# baseline (speedup 1.0000x reference)
"""BLT model TRN2 kernel — nn_BLTModel_13872744366807.

Strategy v3:
- Vocab collapse: the byte-axis path depends only on byte VALUE (V=256),
  so the [B,4096,*] byte axis collapses to a [B,256,*] table; patch
  mean-pooling becomes a host-computed histogram matrix times emb; final
  output is a host gather.
- TP-8 (Megatron heads/hidden sharding) with BATCH-PIPELINED execution:
  the two batches are fully independent through the transformer, so each
  sublayer computes batch 0, fires its fp16 AllReduce (512 KB, 8 cores),
  then computes batch 1 while b0's collective flies on the TOPSP/SDMA
  engines. Steady-state: CC pipe ~100% busy, AR latency hidden behind
  the other batch's compute.
- bf16 weights + activation mirrors for all big matmuls (FWL weight
  loads, half DMA); fp32 residual stream + LN stats (f32r matmuls for
  the [1,N] stat reductions/broadcasts); LayerNorm commuted through
  weight matmuls with host-folded affines + colsum fixups (exact).
"""
import numpy as np
import ml_dtypes
import concourse.bacc as bacc
import concourse.bass as bass
import concourse.mybir as mybir
from concourse import tile
from concourse.bass_utils import run_bass_kernel_spmd
from concourse.bass_interp import get_hw_module

F32 = mybir.dt.float32
F32R = mybir.dt.float32r
BF16 = mybir.dt.bfloat16
FP16 = mybir.dt.float16
AF = mybir.ActivationFunctionType
ALU = mybir.AluOpType
BF = ml_dtypes.bfloat16

L, B, S, P, H, V, NC = 4, 2, 4096, 256, 1024, 256, 8
EPS = 1e-6
RG = [list(range(NC))]

_CACHE = {}


# --------------------------------------------------------------------------
# device program
# --------------------------------------------------------------------------
def _trace(skip_kvn_ln):
    nc = bacc.Bacc("TRN2", target_bir_lowering=False, debug=False,
                   num_devices=NC)
    d = {}

    def inp(name, shape, dt=BF16):
        d[name] = nc.dram_tensor(name, shape, dt, kind="ExternalInput").ap()

    inp("wqkv", [L, 128, 3072])
    inp("wsq", [L, 128, 3], F32)
    inp("ngq", [L, 128, 3], F32)
    inp("wo", [L, 128, 1024])
    inp("bo8", [L, 128, 8], F32)
    inp("w1", [L, 128, 4096])
    inp("ws1", [L, 128, 4], F32)
    inp("ng1", [L, 128, 4], F32)
    inp("w2", [L, 128, 4096])
    inp("b28", [L, 128, 8], F32)
    inp("wq", [128, 1024]); inp("wk", [128, 1024]); inp("wv", [128, 1024])
    inp("bq", [128, 1], F32); inp("bk", [128, 1], F32); inp("bv", [128, 1], F32)
    inp("cawoT", [128, 1024])
    inp("headw", [128, 2048])
    inp("headb", [128, 2], F32)
    inp("embT", [128, 2048])
    inp("embS", [128, 2048])
    inp("cnt", [128, 1024])
    inp("masks", [128, 512])
    inp("ones16", [128, 128])
    inp("onesr", [128, 128], F32R)
    inp("ident", [128, 128])
    inp("fng", [128, 8], F32); inp("fnb", [128, 8], F32)
    inp("cag", [128, 8], F32); inp("cab", [128, 8], F32)
    out_d = nc.dram_tensor("ltab", [128, 1024], F32, kind="ExternalOutput").ap()
    ltab_v = out_d.rearrange("p (lt b x) -> p lt b x", lt=2, b=2)

    with tile.TileContext(nc) as tc:
        with (
            tc.tile_pool(name="const", bufs=1) as cp,
            tc.tile_pool(name="sb", bufs=1) as sbp,
            tc.tile_pool(name="wts", bufs=2) as wp,
            tc.tile_pool(name="cwts", bufs=1) as cwp,
            tc.tile_pool(name="tmp", bufs=3) as tp,
            tc.tile_pool(name="tps", bufs=2) as tps,
            tc.tile_pool(name="pp", bufs=3, space="PSUM") as pp,
            tc.tile_pool(name="pa", bufs=3, space="PSUM") as pa,
            tc.tile_pool(name="pst", bufs=2, space="PSUM") as pst,
            tc.tile_pool(name="dram", bufs=1, space="DRAM") as dp,
        ):
            # ---------------- constants ----------------
            def cload(name, shape, dt=BF16):
                t_ = cp.tile(shape, dt, tag=name)
                nc.sync.dma_start(t_[:], d[name][:])
                return t_

            ones16_t = cload("ones16", [128, 128])
            onesr_t = cload("onesr", [128, 128], F32R)
            onesf_t = cp.tile([1, 128], F32, tag="onesf")
            nc.sync.dma_start(onesf_t[:], d["onesr"][0:1, :].bitcast(F32))
            ident_t = cload("ident", [128, 128])
            masks_t = cload("masks", [128, 512])
            fng_t = cload("fng", [128, 8], F32); fnb_t = cload("fnb", [128, 8], F32)
            cag_t = cload("cag", [128, 8], F32); cab_t = cload("cab", [128, 8], F32)
            headb_t = cload("headb", [128, 2], F32)
            bq_t = cload("bq", [128, 1], F32); bk_t = cload("bk", [128, 1], F32)
            bv_t = cload("bv", [128, 1], F32)
            embS_t = cp.tile([128, 2, 1024], BF16, tag="embS")
            nc.sync.dma_start(embS_t[:], d["embS"][:].rearrange(
                "p (vc x) -> p vc x", vc=2))
            cnt_t = cp.tile([128, 2, 2, 256], BF16, tag="cnt")
            nc.sync.dma_start(cnt_t[:], d["cnt"][:].rearrange(
                "p (vc b x) -> p vc b x", vc=2, b=2))

            # cc warm-up: tiny AllReduce to absorb first-call skew
            wbin = dp.tile([128, 8], F32, tag="wrmi")
            wbout = dp.tile([128, 8], F32, addr_space="Shared", tag="wrmo")
            nc.sync.dma_start(wbin[:], d["bo8"][0].bitcast(F32))
            nc.gpsimd.collective_compute(
                "AllReduce", ALU.add, replica_groups=RG,
                ins=[wbin[:].opt()], outs=[wbout[:].opt()])

            # ---------------- persistent activations (per batch) -----------
            h_t = [sbp.tile([128, 8, 256], F32R, name=f"h{b}", tag=f"h{b}") for b in range(2)]
            h16_t = [sbp.tile([128, 8, 256], BF16, name=f"h16_{b}", tag=f"h16_{b}")
                     for b in range(2)]
            sq_t = [sbp.tile([128, 8, 256], F32R, name=f"sq{b}", tag=f"sq{b}")
                    for b in range(2)]
            qkv_t = [sbp.tile([128, 3, 256], BF16, name=f"qkv{b}", tag=f"qkv{b}")
                     for b in range(2)]
            qkvh2_t = [sbp.tile([64, 3, 256], BF16, name=f"qkvh2_{b}", tag=f"qkvh2_{b}")
                       for b in range(2)]
            A_t = [sbp.tile([128, 256], BF16, name=f"A{b}", tag=f"A{b}") for b in range(2)]
            gu_t = [sbp.tile([128, 4, 256], BF16, name=f"gu{b}", tag=f"gu{b}")
                    for b in range(2)]
            aro_t = [sbp.tile([128, 8, 256], FP16, name=f"aro{b}", tag=f"aro{b}")
                     for b in range(2)]
            ari_t = [sbp.tile([128, 8, 256], FP16, name=f"ari{b}", tag=f"ari{b}")
                     for b in range(2)]

            # ---------------- helpers ----------------
            def stats(src, srcdt, b):
                """src: [128, 8, 256]. Returns (rsig_b, musig_b)
                [128, 256] f32r, broadcast across partitions."""
                ones_src = onesr_t if srcdt == "f32r" else ones16_t
                sq = sq_t[b]
                nc.scalar.activation(sq[:], src[:], AF.Square)
                ps_sum = pst.tile([1, 256], F32, tag="stat")
                ps_sq = pst.tile([1, 256], F32, tag="stat")
                for ti in range(8):
                    nc.tensor.matmul(ps_sum[:], ones_src[:, 0:1],
                                     src[:, ti, :],
                                     start=(ti == 0), stop=(ti == 7))
                for ti in range(8):
                    nc.tensor.matmul(ps_sq[:], onesr_t[:, 0:1],
                                     sq[:, ti, :],
                                     start=(ti == 0), stop=(ti == 7))
                inv = 1.0 / 1024.0
                mu = tps.tile([1, 256], F32R, tag="mu")
                nc.vector.tensor_scalar_mul(mu[:], ps_sum[:], inv)
                ex2 = tps.tile([1, 256], F32R, tag="ex2")
                nc.vector.tensor_scalar(out=ex2[:], in0=ps_sq[:],
                                        scalar1=inv, scalar2=EPS,
                                        op0=ALU.mult, op1=ALU.add)
                mus = tps.tile([1, 256], F32R, tag="mus")
                nc.scalar.activation(mus[:], mu[:], AF.Square)
                vare = tps.tile([1, 256], F32, tag="var")
                nc.vector.tensor_tensor(out=vare[:], in0=ex2[:],
                                        in1=mus[:], op=ALU.subtract)
                vrec = tps.tile([1, 256], F32, tag="vrec")
                nc.vector.reciprocal_approx_fast(out=vrec[:], in_=vare[:])
                rsig = tps.tile([1, 256], F32R, tag="rsig")
                nc.scalar.activation(rsig[:], vrec[:], AF.Sqrt)
                musg = tps.tile([1, 256], F32R, tag="musg")
                nc.vector.tensor_tensor(out=musg[:], in0=mu[:],
                                        in1=rsig[:], op=ALU.mult)
                pb = pp.tile([128, 256], F32, tag="mm")
                nc.tensor.matmul(pb[:], onesr_t[0:1, :], rsig[:],
                                 start=True, stop=True)
                rsig_b = tp.tile([128, 256], F32R, tag="rsigb")
                nc.vector.tensor_copy(rsig_b[:], pb[:])
                pb2 = pp.tile([128, 256], F32, tag="mm")
                nc.tensor.matmul(pb2[:], onesr_t[0:1, :], musg[:],
                                 start=True, stop=True)
                musig_b = tp.tile([128, 256], F32R, tag="musgb")
                nc.vector.tensor_copy(musig_b[:], pb2[:])
                return rsig_b, musig_b

            def fixup(ps, mcol, rsig_b, musig_b, wsum_t, negb_t, out_ap,
                      gelu=False):
                """out = ps*rsig_b - (musig_b*wsum - (-negb)); optional Gelu."""
                t1 = tp.tile([128, 256], F32R, tag="fx1")
                nc.vector.tensor_tensor(out=t1[:], in0=ps[:], in1=rsig_b[:],
                                        op=ALU.mult)
                m2 = tp.tile([128, 256], F32R, tag="fx2")
                nc.vector.tensor_scalar(out=m2[:], in0=musig_b[:],
                                        scalar1=wsum_t[:, mcol:mcol + 1],
                                        scalar2=negb_t[:, mcol:mcol + 1],
                                        op0=ALU.mult, op1=ALU.add)
                if gelu:
                    t2 = tp.tile([128, 256], F32R, tag="fx3")
                    nc.vector.tensor_tensor(out=t2[:], in0=t1[:], in1=m2[:],
                                            op=ALU.subtract)
                    nc.scalar.activation(out_ap, t2[:], AF.Gelu)
                else:
                    nc.vector.tensor_tensor(out=out_ap, in0=t1[:], in1=m2[:],
                                            op=ALU.subtract)

            def allreduce_fp16(tag, b):
                bin_ = dp.tile([128, 2048], FP16, tag=f"ci{tag}")
                bout = dp.tile([128, 2048], FP16, addr_space="Shared",
                               tag=f"co{tag}")
                for q in range(4):
                    nc.sync.dma_start(bin_[:, q * 512:(q + 1) * 512],
                                      aro_t[b][:, q * 2:(q + 1) * 2, :])
                nc.gpsimd.collective_compute(
                    "AllReduce", ALU.add, replica_groups=RG,
                    ins=[bin_[:].opt()], outs=[bout[:].opt()])
                for q in range(8):
                    nc.sync.dma_start(ari_t[b][:, q, :],
                                      bout[:, q * 256:(q + 1) * 256])

            def resid_add(b):
                for ti in range(8):
                    nc.vector.tensor_tensor(out=h_t[b][:, ti, :],
                                            in0=h_t[b][:, ti, :],
                                            in1=ari_t[b][:, ti, :],
                                            op=ALU.add)
                    nc.vector.tensor_copy(h16_t[b][:, ti, :], h_t[b][:, ti, :])

            # ---------------- patch pooling ----------------
            for b in range(2):
                for ti in range(8):
                    ps = pp.tile([128, 256], F32, tag="mm")
                    for vc in range(2):
                        nc.tensor.matmul(
                            ps[:], embS_t[:, vc, ti * 128:(ti + 1) * 128],
                            cnt_t[:, vc, b, :],
                            start=(vc == 0), stop=(vc == 1))
                    nc.vector.tensor_copy(h_t[b][:, ti, :], ps[:])
                    nc.vector.tensor_copy(h16_t[b][:, ti, :], ps[:])

            # ---------------- transformer layers ----------------
            def attn_sublayer(l, b, wqkv_t, wo_t, wsq_t, ngq_t, bo8_t):
                rsb, msb = stats(h_t[b], "f32r", b)
                for j in range(3):
                    ps = pp.tile([128, 256], F32, tag="mm")
                    for kc in range(8):
                        nc.tensor.matmul(
                            ps[:], wqkv_t[:, kc, j * 128:(j + 1) * 128],
                            h16_t[b][:, kc, :],
                            start=(kc == 0), stop=(kc == 7))
                    fixup(ps, j, rsb, msb, wsq_t, ngq_t, qkv_t[b][:, j, :])
                # shift upper-head rows (partitions 64-127) down to base 0
                nc.sync.dma_start(qkvh2_t[b][:], qkv_t[b][64:128, :, :])

                for hh in range(2):
                    src = qkv_t[b] if hh == 0 else qkvh2_t[b]
                    qT = src[0:64, 0, :]
                    kT = src[0:64, 1, :]
                    vT = src[0:64, 2, :]
                    em = tp.tile([128, 2, 256], BF16, tag="em")
                    for kt in range(2):
                        ps_s = pa.tile([128, 256], F32, tag="att")
                        nc.tensor.matmul(ps_s[:],
                                         kT[:, kt * 128:(kt + 1) * 128],
                                         qT[:], start=True, stop=True)
                        ex = tp.tile([128, 256], BF16, tag="ex")
                        nc.scalar.activation(ex[:], ps_s[:], AF.Exp,
                                             scale=0.125)
                        nc.vector.tensor_tensor(
                            out=em[:, kt, :], in0=ex[:],
                            in1=masks_t[:, kt * 256:(kt + 1) * 256],
                            op=ALU.mult)
                    ps_d = pst.tile([1, 256], F32, tag="stat")
                    for kt in range(2):
                        nc.tensor.matmul(ps_d[:], ones16_t[:, 0:1],
                                         em[:, kt, :],
                                         start=(kt == 0), stop=(kt == 1))
                    rec = tps.tile([1, 256], F32, tag="rec")
                    nc.vector.reciprocal_approx_fast(out=rec[:], in_=ps_d[:])
                    ps_rb = pp.tile([128, 256], F32, tag="mm")
                    nc.tensor.matmul(ps_rb[:], onesf_t[0:1, :], rec[:],
                                     start=True, stop=True)
                    rec_b = tp.tile([128, 256], F32R, tag="recb")
                    nc.vector.tensor_copy(rec_b[:], ps_rb[:])
                    vtok = tp.tile([128, 2, 64], BF16, tag="vtok")
                    for kt in range(2):
                        ps_t = pa.tile([128, 256], BF16, tag="att")
                        nc.tensor.transpose(ps_t[:, :64],
                                            vT[:, kt * 128:(kt + 1) * 128],
                                            ident_t[0:64, 0:64])
                        nc.vector.tensor_copy(vtok[:, kt, :], ps_t[:, :64])
                    ps_o = pa.tile([128, 256], F32, tag="att")
                    for kt in range(2):
                        nc.tensor.matmul(
                            ps_o[0:64, :], vtok[:, kt, :],
                            em[:, kt, :], start=(kt == 0), stop=(kt == 1))
                    if hh == 0:
                        nc.vector.tensor_tensor(
                            out=A_t[b][0:64, :], in0=ps_o[0:64, :],
                            in1=rec_b[0:64, :], op=ALU.mult)
                    else:
                        oh = tp.tile([64, 256], BF16, tag="oh")
                        nc.vector.tensor_tensor(
                            out=oh[:], in0=ps_o[0:64, :],
                            in1=rec_b[0:64, :], op=ALU.mult)
                        nc.sync.dma_start(A_t[b][64:128, :], oh[:])

                for m in range(8):
                    ps = pp.tile([128, 256], F32, tag="mm")
                    nc.tensor.matmul(ps[:], wo_t[:, m * 128:(m + 1) * 128],
                                     A_t[b][:], start=True, stop=True)
                    nc.vector.tensor_scalar(out=aro_t[b][:, m, :], in0=ps[:],
                                            scalar1=bo8_t[:, m:m + 1],
                                            scalar2=None, op0=ALU.add)
                allreduce_fp16(f"a{l}b{b}", b)

            def mlp_sublayer(l, b, w1_t, w2_t, ws1_t, ng1_t, b28_t):
                rsb, msb = stats(h_t[b], "f32r", b)
                for m in range(4):
                    ps = pp.tile([128, 256], F32, tag="mm")
                    for kc in range(8):
                        nc.tensor.matmul(
                            ps[:], w1_t[:, kc, m * 128:(m + 1) * 128],
                            h16_t[b][:, kc, :],
                            start=(kc == 0), stop=(kc == 7))
                    fixup(ps, m, rsb, msb, ws1_t, ng1_t, gu_t[b][:, m, :],
                          gelu=True)
                for m in range(8):
                    ps = pp.tile([128, 256], F32, tag="mm")
                    for kc in range(4):
                        nc.tensor.matmul(
                            ps[:], w2_t[:, kc, m * 128:(m + 1) * 128],
                            gu_t[b][:, kc, :],
                            start=(kc == 0), stop=(kc == 3))
                    nc.vector.tensor_scalar(out=aro_t[b][:, m, :], in0=ps[:],
                                            scalar1=b28_t[:, m:m + 1],
                                            scalar2=None, op0=ALU.add)
                allreduce_fp16(f"m{l}b{b}", b)

            for l in range(4):
                wqkv_t = wp.tile([128, 8, 384], BF16, tag="wqkv")
                for q in range(2):
                    nc.sync.dma_start(
                        wqkv_t[:, q * 4:(q + 1) * 4, :],
                        d["wqkv"][l].rearrange("p (kc x) -> p kc x", kc=8)
                        [:, q * 4:(q + 1) * 4, :])
                wo_t = wp.tile([128, 1024], BF16, tag="wo")
                nc.sync.dma_start(wo_t[:], d["wo"][l])
                wsq_t = wp.tile([128, 3], F32, tag="wsq")
                nc.sync.dma_start(wsq_t[:], d["wsq"][l])
                ngq_t = wp.tile([128, 3], F32, tag="ngq")
                nc.sync.dma_start(ngq_t[:], d["ngq"][l])
                bo8_t = wp.tile([128, 8], F32, tag="bo8")
                nc.sync.dma_start(bo8_t[:], d["bo8"][l])
                w1_t = wp.tile([128, 8, 512], BF16, tag="w1")
                for q in range(2):
                    nc.sync.dma_start(
                        w1_t[:, q * 4:(q + 1) * 4, :],
                        d["w1"][l].rearrange("p (kc x) -> p kc x", kc=8)
                        [:, q * 4:(q + 1) * 4, :])
                w2_t = wp.tile([128, 4, 1024], BF16, tag="w2")
                for q in range(2):
                    nc.sync.dma_start(
                        w2_t[:, q * 2:(q + 1) * 2, :],
                        d["w2"][l].rearrange("p (kc x) -> p kc x", kc=4)
                        [:, q * 2:(q + 1) * 2, :])
                ws1_t = wp.tile([128, 4], F32, tag="ws1")
                nc.sync.dma_start(ws1_t[:], d["ws1"][l])
                ng1_t = wp.tile([128, 4], F32, tag="ng1")
                nc.sync.dma_start(ng1_t[:], d["ng1"][l])
                b28_t = wp.tile([128, 8], F32, tag="b28")
                nc.sync.dma_start(b28_t[:], d["b28"][l])

                for b in range(2):
                    if l > 0:
                        resid_add(b)        # previous layer's mlp AR
                    attn_sublayer(l, b, wqkv_t, wo_t, wsq_t, ngq_t, bo8_t)
                for b in range(2):
                    resid_add(b)            # attn AR
                    mlp_sublayer(l, b, w1_t, w2_t, ws1_t, ng1_t, b28_t)

            # ---------------- tail: final norm, CA, logits ----------------
            embT_t = sbp.tile([128, 8, 256], BF16, tag="embT")
            nc.sync.dma_start(embT_t[:], d["embT"][:].rearrange(
                "p (kc x) -> p kc x", kc=8))
            headw_t = sbp.tile([128, 8, 256], BF16, tag="headw")
            nc.sync.dma_start(headw_t[:], d["headw"][:].rearrange(
                "p (kc x) -> p kc x", kc=8))
            wq_t = cwp.tile([128, 8, 128], BF16, tag="wqca")
            nc.sync.dma_start(wq_t[:], d["wq"][:].rearrange(
                "p (kc x) -> p kc x", kc=8))
            wk_t = cwp.tile([128, 8, 128], BF16, tag="wkca")
            nc.sync.dma_start(wk_t[:], d["wk"][:].rearrange(
                "p (kc x) -> p kc x", kc=8))
            wv_t = cwp.tile([128, 8, 128], BF16, tag="wvca")
            nc.sync.dma_start(wv_t[:], d["wv"][:].rearrange(
                "p (kc x) -> p kc x", kc=8))
            cawoT_t = cwp.tile([128, 8, 128], BF16, tag="cawoT")
            nc.sync.dma_start(cawoT_t[:], d["cawoT"][:].rearrange(
                "p (kc x) -> p kc x", kc=8))

            # qn = ln(embT)*cag + cab (shared between batches)
            qn_t = sbp.tile([128, 8, 256], BF16, tag="qn")
            rsb, msb = stats(embT_t, "bf16", 0)
            for ti in range(8):
                t1 = tp.tile([128, 256], F32R, tag="fx1")
                nc.vector.tensor_tensor(out=t1[:], in0=embT_t[:, ti, :],
                                        in1=rsb[:], op=ALU.mult)
                t2 = tp.tile([128, 256], F32R, tag="fx2")
                nc.vector.tensor_tensor(out=t2[:], in0=t1[:], in1=msb[:],
                                        op=ALU.subtract)
                nc.vector.tensor_scalar(out=qn_t[:, ti, :], in0=t2[:],
                                        scalar1=cag_t[:, ti:ti + 1],
                                        scalar2=cab_t[:, ti:ti + 1],
                                        op0=ALU.mult, op1=ALU.add)

            # qT (shared vocab queries)
            qT_t = sbp.tile([128, 256], BF16, tag="qT")
            ps = pp.tile([128, 256], F32, tag="mm")
            for kc in range(8):
                nc.tensor.matmul(ps[:], wq_t[:, kc, :], qn_t[:, kc, :],
                                 start=(kc == 0), stop=(kc == 7))
            nc.vector.tensor_scalar(out=qT_t[:], in0=ps[:],
                                    scalar1=bq_t[:], scalar2=None,
                                    op0=ALU.add)

            # w2c = cawoT.T @ headw (shared) and emb@head_w term (shared)
            w2c_t = sbp.tile([128, 256], BF16, tag="w2c")
            ps = pp.tile([128, 256], F32, tag="mm")
            for kc in range(8):
                nc.tensor.matmul(ps[:], cawoT_t[:, kc, :], headw_t[:, kc, :],
                                 start=(kc == 0), stop=(kc == 7))
            nc.vector.tensor_copy(w2c_t[:], ps[:])
            et_t = sbp.tile([128, 2, 256], F32, tag="et")
            for lt in range(2):
                ps_e = pp.tile([128, 256], F32, tag="mm")
                for kc in range(8):
                    nc.tensor.matmul(ps_e[:],
                                     headw_t[:, kc, lt * 128:(lt + 1) * 128],
                                     embT_t[:, kc, :],
                                     start=(kc == 0), stop=(kc == 7))
                nc.vector.tensor_copy(et_t[:, lt, :], ps_e[:])

            # per-batch: final norm -> kvn, CA, logits partial, AR
            kvn_t = [None, None]
            lar_t = [sbp.tile([128, 2, 256], FP16, name=f"lar{b}", tag=f"lar{b}")
                     for b in range(2)]
            for b in range(2):
                resid_add(b)                # last mlp AR
                rsb, msb = stats(h_t[b], "f32r", b)
                for ti in range(8):
                    t1 = tp.tile([128, 256], F32R, tag="fx1")
                    nc.vector.tensor_tensor(out=t1[:], in0=h_t[b][:, ti, :],
                                            in1=rsb[:], op=ALU.mult)
                    t2 = tp.tile([128, 256], F32R, tag="fx2")
                    nc.vector.tensor_tensor(out=t2[:], in0=t1[:], in1=msb[:],
                                            op=ALU.subtract)
                    nc.vector.tensor_scalar(out=h16_t[b][:, ti, :], in0=t2[:],
                                            scalar1=fng_t[:, ti:ti + 1],
                                            scalar2=fnb_t[:, ti:ti + 1],
                                            op0=ALU.mult, op1=ALU.add)
                if skip_kvn_ln:
                    kvn_t[b] = h16_t[b]
                else:
                    kvn_t[b] = sbp.tile([128, 8, 256], BF16, tag=f"kvn{b}")
                    rsb, msb = stats(h16_t[b], "bf16", b)
                    for ti in range(8):
                        t1 = tp.tile([128, 256], F32R, tag="fx1")
                        nc.vector.tensor_tensor(out=t1[:],
                                                in0=h16_t[b][:, ti, :],
                                                in1=rsb[:], op=ALU.mult)
                        t2 = tp.tile([128, 256], F32R, tag="fx2")
                        nc.vector.tensor_tensor(out=t2[:], in0=t1[:],
                                                in1=msb[:], op=ALU.subtract)
                        nc.vector.tensor_scalar(out=kvn_t[b][:, ti, :],
                                                in0=t2[:],
                                                scalar1=cag_t[:, ti:ti + 1],
                                                scalar2=cab_t[:, ti:ti + 1],
                                                op0=ALU.mult, op1=ALU.add)

                kT_t = sbp.tile([128, 256], BF16, tag=f"kT{b}")
                vT_t = sbp.tile([128, 256], BF16, tag=f"vT{b}")
                for (w_v, bias_t, out_t) in ((wk_t, bk_t, kT_t),
                                             (wv_t, bv_t, vT_t)):
                    ps = pp.tile([128, 256], F32, tag="mm")
                    for kc in range(8):
                        nc.tensor.matmul(ps[:], w_v[:, kc, :],
                                         kvn_t[b][:, kc, :],
                                         start=(kc == 0), stop=(kc == 7))
                    nc.vector.tensor_scalar(out=out_t[:], in0=ps[:],
                                            scalar1=bias_t[:], scalar2=None,
                                            op0=ALU.add)

                em = tp.tile([128, 2, 256], BF16, tag="em")
                for kt in range(2):
                    ps_s = pa.tile([128, 256], F32, tag="att")
                    nc.tensor.matmul(
                        ps_s[:], kT_t[:, kt * 128:(kt + 1) * 128],
                        qT_t[:], start=True, stop=True)
                    nc.scalar.activation(em[:, kt, :], ps_s[:], AF.Exp,
                                         scale=float(1.0 / np.sqrt(128.0)))
                ps_d = pst.tile([1, 256], F32, tag="stat")
                for kt in range(2):
                    nc.tensor.matmul(ps_d[:], ones16_t[:, 0:1], em[:, kt, :],
                                     start=(kt == 0), stop=(kt == 1))
                rec = tps.tile([1, 256], F32, tag="rec")
                nc.vector.reciprocal_approx_fast(out=rec[:], in_=ps_d[:])
                ps_rb = pp.tile([128, 256], F32, tag="mm")
                nc.tensor.matmul(ps_rb[:], onesf_t[0:1, :], rec[:],
                                 start=True, stop=True)
                rec_b = tp.tile([128, 256], F32R, tag="recb")
                nc.vector.tensor_copy(rec_b[:], ps_rb[:])
                vtok = tp.tile([128, 2, 128], BF16, tag="vtokca")
                for kt in range(2):
                    ps_t = pa.tile([128, 256], BF16, tag="att")
                    nc.tensor.transpose(
                        ps_t[:, :128],
                        vT_t[:, kt * 128:(kt + 1) * 128],
                        ident_t[:])
                    nc.vector.tensor_copy(vtok[:, kt, :], ps_t[:, :128])
                ps_o = pa.tile([128, 256], F32, tag="att")
                for kt in range(2):
                    nc.tensor.matmul(ps_o[:], vtok[:, kt, :], em[:, kt, :],
                                     start=(kt == 0), stop=(kt == 1))
                O_t = tp.tile([128, 256], BF16, tag="O")
                nc.vector.tensor_tensor(out=O_t[:], in0=ps_o[:],
                                        in1=rec_b[:], op=ALU.mult)

                lp_t = sbp.tile([128, 2, 256], FP16, tag=f"lp{b}")
                for lt in range(2):
                    ps = pp.tile([128, 256], F32, tag="mm")
                    nc.tensor.matmul(ps[:],
                                     w2c_t[:, lt * 128:(lt + 1) * 128],
                                     O_t[:], start=True, stop=True)
                    nc.vector.tensor_copy(lp_t[:, lt, :], ps[:])
                lbin = dp.tile([128, 512], FP16, tag=f"lci{b}")
                lbout = dp.tile([128, 512], FP16, addr_space="Shared",
                                tag=f"lco{b}")
                nc.sync.dma_start(lbin[:], lp_t[:])
                nc.gpsimd.collective_compute(
                    "AllReduce", ALU.add, replica_groups=RG,
                    ins=[lbin[:].opt()], outs=[lbout[:].opt()])
                nc.sync.dma_start(lar_t[b][:], lbout[:])

            for b in range(2):
                out_t = sbp.tile([128, 2, 256], F32, tag=f"outt{b}")
                for lt in range(2):
                    tb = tp.tile([128, 256], F32, tag="tb")
                    nc.vector.tensor_scalar(out=tb[:], in0=lar_t[b][:, lt, :],
                                            scalar1=headb_t[:, lt:lt + 1],
                                            scalar2=None, op0=ALU.add)
                    nc.vector.tensor_tensor(out=out_t[:, lt, :],
                                            in0=tb[:], in1=et_t[:, lt, :],
                                            op=ALU.add)
                nc.sync.dma_start(ltab_v[:, :, b, :], out_t[:])

    nc.compile()
    nc.m = get_hw_module(nc.m)
    return nc


# --------------------------------------------------------------------------
# host side
# --------------------------------------------------------------------------
def _shuf(M):
    """[K, X] -> [128, (K//128)*X] laid out as [p, kc, x]."""
    K, X = M.shape
    return np.ascontiguousarray(
        M.reshape(K // 128, 128, X).transpose(1, 0, 2).reshape(128, -1))


def _bf(M):
    return np.ascontiguousarray(M).astype(BF)


def _prep(inputs):
    f = lambda k: np.asarray(inputs[k], np.float32)
    byte_seq = np.asarray(inputs["byte_seq"])
    bd = np.asarray(inputs["patch_boundaries"])
    emb = f("emb")

    # patch histogram matrix
    pos = np.arange(S)
    pid = np.stack([np.searchsorted(bd[b], pos, side="right") for b in range(B)])
    pid = np.clip(pid, 0, P - 1)
    Cn = np.zeros((B, P, V), np.float32)
    for b in range(B):
        np.add.at(Cn[b], (pid[b], byte_seq[b]), 1.0)
    cnts = Cn.sum(-1)
    Cn /= np.maximum(cnts, 1.0)[..., None]
    cnt_all = np.concatenate([Cn[0].T, Cn[1].T], axis=1)  # [V, 512]

    g1, b1a = f("g_ln1_g"), f("g_ln1_b")
    g2, b2a = f("g_ln2_g"), f("g_ln2_b")
    Wqkv, bqkv = f("g_wqkv"), f("g_bqkv")
    Wo, bo = f("g_wo"), f("g_bo")
    W1, b1 = f("g_w1"), f("g_b1")
    W2, b2 = f("g_w2"), f("g_b2")

    Wq_f = g1[:, :, None] * Wqkv                       # [L, H, 3H]
    biasq = np.einsum("lh,lho->lo", b1a, Wqkv) + bqkv  # [L, 3H]
    wsumq = Wq_f.sum(1)                                # [L, 3H]
    W1_f = g2[:, :, None] * W1
    bias1 = np.einsum("lh,lho->lo", b2a, W1) + b1
    wsum1 = W1_f.sum(1)

    ca_wqkv, ca_bqkv = f("ca_wqkv"), f("ca_bqkv")
    ca_wo, ca_bo = f("ca_wo"), f("ca_bo")
    head_w, head_b = f("head_w"), f("head_b")
    headb_full = head_b + ca_bo @ head_w               # [256]

    masks = np.zeros((128, 2, 256), np.float32)
    for kt in range(2):
        ktg = kt * 128 + np.arange(128)
        masks[:, kt, :] = (ktg[:, None] <= np.arange(256)[None, :])

    shared = {
        "headw": _bf(_shuf(head_w)),
        "headb": np.ascontiguousarray(headb_full.reshape(2, 128).T),
        "embT": _bf(_shuf(np.ascontiguousarray(emb.T))),
        "embS": _bf(_shuf(emb)),
        "cnt": _bf(_shuf(cnt_all)),
        "masks": _bf(masks.reshape(128, 512)),
        "ones16": _bf(np.ones((128, 128), np.float32)),
        "onesr": np.ones((128, 128), np.float32),
        "ident": _bf(np.eye(128, dtype=np.float32)),
        "fng": np.ascontiguousarray(f("fn_g").reshape(8, 128).T),
        "fnb": np.ascontiguousarray(f("fn_b").reshape(8, 128).T),
        "cag": np.ascontiguousarray(f("ca_ln_g").reshape(8, 128).T),
        "cab": np.ascontiguousarray(f("ca_ln_b").reshape(8, 128).T),
        "bo8": np.ascontiguousarray(
            bo.reshape(L, 8, 128).transpose(0, 2, 1) / NC),
        "b28": np.ascontiguousarray(
            b2.reshape(L, 8, 128).transpose(0, 2, 1) / NC),
    }

    in_maps = []
    for c in range(NC):
        cols = np.concatenate([np.arange(c * 128, (c + 1) * 128) + k * H
                               for k in range(3)])
        m = dict(shared)
        m["wqkv"] = _bf(np.stack([_shuf(Wq_f[l][:, cols]) for l in range(L)]))
        m["wsq"] = np.ascontiguousarray(
            wsumq[:, cols].reshape(L, 3, 128).transpose(0, 2, 1))
        m["ngq"] = np.ascontiguousarray(
            (-biasq[:, cols]).reshape(L, 3, 128).transpose(0, 2, 1))
        m["wo"] = _bf(Wo[:, c * 128:(c + 1) * 128, :])
        m["w1"] = _bf(np.stack([_shuf(W1_f[l][:, c * 512:(c + 1) * 512])
                                for l in range(L)]))
        m["ws1"] = np.ascontiguousarray(
            wsum1[:, c * 512:(c + 1) * 512].reshape(L, 4, 128)
            .transpose(0, 2, 1))
        m["ng1"] = np.ascontiguousarray(
            (-bias1[:, c * 512:(c + 1) * 512]).reshape(L, 4, 128)
            .transpose(0, 2, 1))
        m["w2"] = _bf(np.stack([_shuf(W2[l][c * 512:(c + 1) * 512, :])
                                for l in range(L)]))
        m["wq"] = _bf(_shuf(ca_wqkv[:, c * 128:(c + 1) * 128]))
        m["wk"] = _bf(_shuf(ca_wqkv[:, H + c * 128: H + (c + 1) * 128]))
        m["wv"] = _bf(_shuf(ca_wqkv[:, 2 * H + c * 128: 2 * H + (c + 1) * 128]))
        m["bq"] = np.ascontiguousarray(
            ca_bqkv[c * 128:(c + 1) * 128, None])
        m["bk"] = np.ascontiguousarray(
            ca_bqkv[H + c * 128: H + (c + 1) * 128, None])
        m["bv"] = np.ascontiguousarray(
            ca_bqkv[2 * H + c * 128: 2 * H + (c + 1) * 128, None])
        m["cawoT"] = _bf(_shuf(np.ascontiguousarray(
            ca_wo[c * 128:(c + 1) * 128, :].T)))
        in_maps.append(m)
    return in_maps, byte_seq


def run_device(inputs, trace=False):
    skip = (np.allclose(np.asarray(inputs["fn_g"]), 1.0)
            and np.allclose(np.asarray(inputs["fn_b"]), 0.0)
            and np.allclose(np.asarray(inputs["ca_ln_g"]), 1.0)
            and np.allclose(np.asarray(inputs["ca_ln_b"]), 0.0))
    key = ("nc", skip)
    if key not in _CACHE:
        _CACHE[key] = _trace(skip)
    nc = _CACHE[key]
    in_maps, byte_seq = _prep(inputs)
    res = run_bass_kernel_spmd(nc, in_maps, core_ids=list(range(NC)),
                               trace=trace)
    ltab = res.results[0]["ltab"]                     # [128, 1024]
    ltab = ltab.reshape(128, 2, 2, 256)
    out = np.empty((B, S, V), np.float32)
    for b in range(B):
        tab = ltab[:, :, b, :].transpose(1, 0, 2).reshape(256, 256)
        out[b] = tab.T[byte_seq[b]]                   # [S, 256]
    return out, res


def kernel(**inputs) -> np.ndarray:
    out, _ = run_device(inputs, trace=False)
    return out


# revision 13
# speedup vs baseline: 1.0054x; 1.0054x over previous
"""BLT model TRN2 kernel — nn_BLTModel_13872744366807.

Strategy v3:
- Vocab collapse: the byte-axis path depends only on byte VALUE (V=256),
  so the [B,4096,*] byte axis collapses to a [B,256,*] table; patch
  mean-pooling becomes a host-computed histogram matrix times emb; final
  output is a host gather.
- TP-8 (Megatron heads/hidden sharding) with BATCH-PIPELINED execution:
  the two batches are fully independent through the transformer, so each
  sublayer computes batch 0, fires its fp16 AllReduce (512 KB, 8 cores),
  then computes batch 1 while b0's collective flies on the TOPSP/SDMA
  engines. Steady-state: CC pipe ~100% busy, AR latency hidden behind
  the other batch's compute.
- bf16 weights + activation mirrors for all big matmuls (FWL weight
  loads, half DMA); fp32 residual stream + LN stats (f32r matmuls for
  the [1,N] stat reductions/broadcasts); LayerNorm commuted through
  weight matmuls with host-folded affines + colsum fixups (exact).
"""
import numpy as np
import ml_dtypes
import concourse.bacc as bacc
import concourse.bass as bass
import concourse.mybir as mybir
from concourse import tile
from concourse.bass_utils import run_bass_kernel_spmd
from concourse.bass_interp import get_hw_module

F32 = mybir.dt.float32
F32R = mybir.dt.float32r
BF16 = mybir.dt.bfloat16
FP16 = mybir.dt.float16
AF = mybir.ActivationFunctionType
ALU = mybir.AluOpType
BF = ml_dtypes.bfloat16

L, B, S, P, H, V, NC = 4, 2, 4096, 256, 1024, 256, 8
EPS = 1e-6
RG = [list(range(NC))]

_CACHE = {}


# --------------------------------------------------------------------------
# device program
# --------------------------------------------------------------------------
def _trace(skip_kvn_ln):
    nc = bacc.Bacc("TRN2", target_bir_lowering=False, debug=False,
                   num_devices=NC)
    d = {}

    def inp(name, shape, dt=BF16):
        d[name] = nc.dram_tensor(name, shape, dt, kind="ExternalInput").ap()

    inp("wqkv", [L, 128, 3072])
    inp("wsq", [L, 128, 3], F32)
    inp("ngq", [L, 128, 3], F32)
    inp("wo", [L, 64, 2048])
    inp("bo8", [L, 128, 8], F32)
    inp("w1", [L, 128, 4096])
    inp("ws1", [L, 128, 4], F32)
    inp("ng1", [L, 128, 4], F32)
    inp("w2", [L, 128, 4096])
    inp("b28", [L, 128, 8], F32)
    inp("wq", [128, 1024]); inp("wk", [128, 1024]); inp("wv", [128, 1024])
    inp("bq", [128, 1], F32); inp("bk", [128, 1], F32); inp("bv", [128, 1], F32)
    inp("cawoT", [128, 1024])
    inp("headw", [128, 2048])
    inp("headb", [128, 2], F32)
    inp("embT", [128, 2048])
    inp("embS", [128, 2048])
    inp("cnt", [128, 1024])
    inp("masks", [128, 512])
    inp("ones16", [128, 128])
    inp("onesr", [128, 128], F32R)
    inp("ident", [128, 128])
    inp("fng", [128, 8], F32); inp("fnb", [128, 8], F32)
    inp("cag", [128, 8], F32); inp("cab", [128, 8], F32)
    out_d = nc.dram_tensor("ltab", [128, 1024], F32, kind="ExternalOutput").ap()
    ltab_v = out_d.rearrange("p (lt b x) -> p lt b x", lt=2, b=2)

    with tile.TileContext(nc) as tc:
        with (
            tc.tile_pool(name="const", bufs=1) as cp,
            tc.tile_pool(name="sb", bufs=1) as sbp,
            tc.tile_pool(name="wts", bufs=2) as wp,
            tc.tile_pool(name="cwts", bufs=1) as cwp,
            tc.tile_pool(name="tmp", bufs=3) as tp,
            tc.tile_pool(name="tps", bufs=2) as tps,
            tc.tile_pool(name="pp", bufs=3, space="PSUM") as pp,
            tc.tile_pool(name="pa", bufs=3, space="PSUM") as pa,
            tc.tile_pool(name="pst", bufs=2, space="PSUM") as pst,
            tc.tile_pool(name="dram", bufs=1, space="DRAM") as dp,
        ):
            # cc warm-up: tiny AllReduce to absorb first-call skew.
            # Issued before every other DMA so it isn't stuck in a queue
            # behind megabytes of weight loads.
            wbin = dp.tile([128, 8], F32, tag="wrmi")
            wbout = dp.tile([128, 8], F32, addr_space="Shared", tag="wrmo")
            nc.sync.dma_start(wbin[:], d["bo8"][0].bitcast(F32))
            nc.gpsimd.collective_compute(
                "AllReduce", ALU.add, replica_groups=RG,
                ins=[wbin[:].opt()], outs=[wbout[:].opt()])

            # ---------------- constants ----------------
            def cload(name, shape, dt=BF16):
                t_ = cp.tile(shape, dt, tag=name)
                nc.sync.dma_start(t_[:], d[name][:])
                return t_

            ones16_t = cload("ones16", [128, 128])
            onesr_t = cload("onesr", [128, 128], F32R)
            onesf_t = cp.tile([1, 128], F32, tag="onesf")
            nc.sync.dma_start(onesf_t[:], d["onesr"][0:1, :].bitcast(F32))
            ident_t = cload("ident", [128, 128])
            masks_t = cload("masks", [128, 512])
            fng_t = cload("fng", [128, 8], F32); fnb_t = cload("fnb", [128, 8], F32)
            cag_t = cload("cag", [128, 8], F32); cab_t = cload("cab", [128, 8], F32)
            headb_t = cload("headb", [128, 2], F32)
            bq_t = cload("bq", [128, 1], F32); bk_t = cload("bk", [128, 1], F32)
            bv_t = cload("bv", [128, 1], F32)
            embS_t = cp.tile([128, 2, 1024], BF16, tag="embS")
            nc.sync.dma_start(embS_t[:], d["embS"][:].rearrange(
                "p (vc x) -> p vc x", vc=2))
            cnt_t = cp.tile([128, 2, 2, 256], BF16, tag="cnt")
            nc.sync.dma_start(cnt_t[:], d["cnt"][:].rearrange(
                "p (vc b x) -> p vc b x", vc=2, b=2))

            # ---------------- persistent activations (per batch) -----------
            h_t = [sbp.tile([128, 8, 256], F32R, name=f"h{b}", tag=f"h{b}") for b in range(2)]
            h16_t = [sbp.tile([128, 8, 256], BF16, name=f"h16_{b}", tag=f"h16_{b}")
                     for b in range(2)]
            sq_t = [sbp.tile([128, 8, 256], F32R, name=f"sq{b}", tag=f"sq{b}")
                    for b in range(2)]
            qkv_t = [sbp.tile([128, 3, 256], BF16, name=f"qkv{b}", tag=f"qkv{b}")
                     for b in range(2)]
            qkvh2_t = [sbp.tile([64, 3, 256], BF16, name=f"qkvh2_{b}", tag=f"qkvh2_{b}")
                       for b in range(2)]
            A_t = [sbp.tile([64, 2, 256], BF16, name=f"A{b}", tag=f"A{b}") for b in range(2)]
            gu_t = [sbp.tile([128, 4, 256], BF16, name=f"gu{b}", tag=f"gu{b}")
                    for b in range(2)]
            aro_t = [sbp.tile([128, 8, 256], FP16, name=f"aro{b}", tag=f"aro{b}")
                     for b in range(2)]
            ari_t = [sbp.tile([128, 8, 256], FP16, name=f"ari{b}", tag=f"ari{b}")
                     for b in range(2)]

            # ---------------- helpers ----------------
            def stats(src, srcdt, b):
                """src: [128, 8, 256]. Returns (rsig_b, musig_b)
                [128, 256] f32r, broadcast across partitions."""
                ones_src = onesr_t if srcdt == "f32r" else ones16_t
                sq = sq_t[b]
                ps_sum = pst.tile([1, 256], F32, tag="stat")
                ps_sq = pst.tile([1, 256], F32, tag="stat")
                # per-chunk squares so the sumsq reduction starts before the
                # residual adds of later chunks have finished
                for ti in range(8):
                    nc.scalar.activation(sq[:, ti, :], src[:, ti, :],
                                         AF.Square)
                for ti in range(8):
                    nc.tensor.matmul(ps_sum[:], ones_src[:, 0:1],
                                     src[:, ti, :],
                                     start=(ti == 0), stop=(ti == 7))
                for ti in range(8):
                    nc.tensor.matmul(ps_sq[:], onesr_t[:, 0:1],
                                     sq[:, ti, :],
                                     start=(ti == 0), stop=(ti == 7))
                inv = 1.0 / 1024.0
                mu = tps.tile([1, 256], F32R, tag="mu")
                nc.vector.tensor_scalar_mul(mu[:], ps_sum[:], inv)
                ex2 = tps.tile([1, 256], F32R, tag="ex2")
                nc.vector.tensor_scalar(out=ex2[:], in0=ps_sq[:],
                                        scalar1=inv, scalar2=EPS,
                                        op0=ALU.mult, op1=ALU.add)
                mus = tps.tile([1, 256], F32R, tag="mus")
                nc.scalar.activation(mus[:], mu[:], AF.Square)
                vare = tps.tile([1, 256], F32, tag="var")
                nc.vector.tensor_tensor(out=vare[:], in0=ex2[:],
                                        in1=mus[:], op=ALU.subtract)
                vrec = tps.tile([1, 256], F32, tag="vrec")
                nc.vector.reciprocal_approx_fast(out=vrec[:], in_=vare[:])
                rsig = tps.tile([1, 256], F32R, tag="rsig")
                nc.scalar.activation(rsig[:], vrec[:], AF.Sqrt)
                musg = tps.tile([1, 256], F32R, tag="musg")
                nc.vector.tensor_tensor(out=musg[:], in0=mu[:],
                                        in1=rsig[:], op=ALU.mult)
                pb = pp.tile([128, 256], F32, tag="mm")
                nc.tensor.matmul(pb[:], onesr_t[0:1, :], rsig[:],
                                 start=True, stop=True)
                rsig_b = tp.tile([128, 256], F32R, tag="rsigb")
                nc.vector.tensor_copy(rsig_b[:], pb[:])
                pb2 = pp.tile([128, 256], F32, tag="mm")
                nc.tensor.matmul(pb2[:], onesr_t[0:1, :], musg[:],
                                 start=True, stop=True)
                musig_b = tp.tile([128, 256], F32R, tag="musgb")
                nc.vector.tensor_copy(musig_b[:], pb2[:])
                return rsig_b, musig_b

            def fixup(ps, mcol, rsig_b, musig_b, wsum_t, negb_t, out_ap,
                      gelu=False):
                """out = ps*rsig_b - (musig_b*wsum - (-negb)); optional Gelu."""
                t1 = tp.tile([128, 256], F32R, tag="fx1")
                nc.vector.tensor_tensor(out=t1[:], in0=ps[:], in1=rsig_b[:],
                                        op=ALU.mult)
                m2 = tp.tile([128, 256], F32R, tag="fx2")
                nc.vector.tensor_scalar(out=m2[:], in0=musig_b[:],
                                        scalar1=wsum_t[:, mcol:mcol + 1],
                                        scalar2=negb_t[:, mcol:mcol + 1],
                                        op0=ALU.mult, op1=ALU.add)
                if gelu:
                    t2 = tp.tile([128, 256], F32R, tag="fx3")
                    nc.vector.tensor_tensor(out=t2[:], in0=t1[:], in1=m2[:],
                                            op=ALU.subtract)
                    nc.scalar.activation(out_ap, t2[:], AF.Gelu)
                else:
                    nc.vector.tensor_tensor(out=out_ap, in0=t1[:], in1=m2[:],
                                            op=ALU.subtract)

            def allreduce_fp16(tag, b):
                bin_ = dp.tile([128, 2048], FP16, tag=f"ci{tag}")
                bout = dp.tile([128, 2048], FP16, addr_space="Shared",
                               tag=f"co{tag}")
                for q in range(8):
                    nc.sync.dma_start(bin_[:, q * 256:(q + 1) * 256],
                                      aro_t[b][:, q, :])
                nc.gpsimd.collective_compute(
                    "AllReduce", ALU.add, replica_groups=RG,
                    ins=[bin_[:].opt()], outs=[bout[:].opt()])
                for q in range(8):
                    nc.sync.dma_start(ari_t[b][:, q, :],
                                      bout[:, q * 256:(q + 1) * 256])

            def resid_add(b):
                for ti in range(8):
                    nc.vector.tensor_tensor(out=h_t[b][:, ti, :],
                                            in0=h_t[b][:, ti, :],
                                            in1=ari_t[b][:, ti, :],
                                            op=ALU.add)
                    nc.scalar.activation(h16_t[b][:, ti, :], h_t[b][:, ti, :],
                                         AF.Copy)

            # ---------------- patch pooling ----------------
            for b in range(2):
                for ti in range(8):
                    ps = pp.tile([128, 256], F32, tag="mm")
                    for vc in range(2):
                        nc.tensor.matmul(
                            ps[:], embS_t[:, vc, ti * 128:(ti + 1) * 128],
                            cnt_t[:, vc, b, :],
                            start=(vc == 0), stop=(vc == 1))
                    nc.vector.tensor_copy(h_t[b][:, ti, :], ps[:])
                    nc.vector.tensor_copy(h16_t[b][:, ti, :], ps[:])

            # ---------------- transformer layers ----------------
            def attn_sublayer(l, b, wqkv_t, wo_t, wsq_t, ngq_t, bo8_t):
                rsb, msb = stats(h_t[b], "f32r", b)
                for j in range(3):
                    ps = pp.tile([128, 256], F32, tag="mm")
                    for kc in range(8):
                        nc.tensor.matmul(
                            ps[:], wqkv_t[:, kc, j * 128:(j + 1) * 128],
                            h16_t[b][:, kc, :],
                            start=(kc == 0), stop=(kc == 7))
                    fixup(ps, j, rsb, msb, wsq_t, ngq_t, qkv_t[b][:, j, :])
                # shift upper-head rows (partitions 64-127) down to base 0
                nc.sync.dma_start(qkvh2_t[b][:], qkv_t[b][64:128, :, :])

                for hh in range(2):
                    src = qkv_t[b] if hh == 0 else qkvh2_t[b]
                    qT = src[0:64, 0, :]
                    kT = src[0:64, 1, :]
                    vT = src[0:64, 2, :]
                    vtok = tp.tile([128, 2, 64], BF16, tag="vtok")
                    for kt in range(2):
                        ps_t = pa.tile([128, 256], BF16, tag="att")
                        nc.tensor.transpose(ps_t[:, :64],
                                            vT[:, kt * 128:(kt + 1) * 128],
                                            ident_t[0:64, 0:64])
                        nc.vector.tensor_copy(vtok[:, kt, :], ps_t[:, :64])
                    em = tp.tile([128, 2, 256], BF16, tag="em")
                    for kt in range(2):
                        ps_s = pa.tile([128, 256], F32, tag="att")
                        nc.tensor.matmul(ps_s[:],
                                         kT[:, kt * 128:(kt + 1) * 128],
                                         qT[:], start=True, stop=True)
                        ex = tp.tile([128, 256], BF16, tag="ex")
                        nc.scalar.activation(ex[:], ps_s[:], AF.Exp,
                                             scale=0.125)
                        nc.vector.tensor_tensor(
                            out=em[:, kt, :], in0=ex[:],
                            in1=masks_t[:, kt * 256:(kt + 1) * 256],
                            op=ALU.mult)
                    ps_d = pst.tile([1, 256], F32, tag="stat")
                    for kt in range(2):
                        nc.tensor.matmul(ps_d[:], ones16_t[:, 0:1],
                                         em[:, kt, :],
                                         start=(kt == 0), stop=(kt == 1))
                    rec = tps.tile([1, 256], F32, tag="rec")
                    nc.vector.reciprocal_approx_fast(out=rec[:], in_=ps_d[:])
                    ps_rb = pp.tile([128, 256], F32, tag="mm")
                    nc.tensor.matmul(ps_rb[:], onesf_t[0:1, :], rec[:],
                                     start=True, stop=True)
                    rec_b = tp.tile([128, 256], F32R, tag="recb")
                    nc.vector.tensor_copy(rec_b[:], ps_rb[:])
                    ps_o = pa.tile([128, 256], F32, tag="att")
                    for kt in range(2):
                        nc.tensor.matmul(
                            ps_o[0:64, :], vtok[:, kt, :],
                            em[:, kt, :], start=(kt == 0), stop=(kt == 1))
                    nc.vector.tensor_tensor(
                        out=A_t[b][:, hh, :], in0=ps_o[0:64, :],
                        in1=rec_b[0:64, :], op=ALU.mult)

                for m in range(8):
                    ps = pp.tile([128, 256], F32, tag="mm")
                    for hh in range(2):
                        nc.tensor.matmul(
                            ps[:], wo_t[0:64, hh, m * 128:(m + 1) * 128],
                            A_t[b][:, hh, :],
                            start=(hh == 0), stop=(hh == 1))
                    nc.vector.tensor_scalar(out=aro_t[b][:, m, :], in0=ps[:],
                                            scalar1=bo8_t[:, m:m + 1],
                                            scalar2=None, op0=ALU.add)
                allreduce_fp16(f"a{l}b{b}", b)

            def mlp_sublayer(l, b, w1_t, w2_t, ws1_t, ng1_t, b28_t):
                rsb, msb = stats(h_t[b], "f32r", b)
                for m in range(4):
                    ps = pp.tile([128, 256], F32, tag="mm")
                    for kc in range(8):
                        nc.tensor.matmul(
                            ps[:], w1_t[:, kc, m * 128:(m + 1) * 128],
                            h16_t[b][:, kc, :],
                            start=(kc == 0), stop=(kc == 7))
                    fixup(ps, m, rsb, msb, ws1_t, ng1_t, gu_t[b][:, m, :],
                          gelu=True)
                for m in range(8):
                    ps = pp.tile([128, 256], F32, tag="mm")
                    for kc in range(4):
                        nc.tensor.matmul(
                            ps[:], w2_t[:, kc, m * 128:(m + 1) * 128],
                            gu_t[b][:, kc, :],
                            start=(kc == 0), stop=(kc == 3))
                    nc.vector.tensor_scalar(out=aro_t[b][:, m, :], in0=ps[:],
                                            scalar1=b28_t[:, m:m + 1],
                                            scalar2=None, op0=ALU.add)
                allreduce_fp16(f"m{l}b{b}", b)

            for l in range(4):
                wqkv_t = wp.tile([128, 8, 384], BF16, tag="wqkv")
                for q in range(2):
                    nc.sync.dma_start(
                        wqkv_t[:, q * 4:(q + 1) * 4, :],
                        d["wqkv"][l].rearrange("p (kc x) -> p kc x", kc=8)
                        [:, q * 4:(q + 1) * 4, :])
                wo_t = wp.tile([64, 2, 1024], BF16, tag="wo")
                nc.sync.dma_start(wo_t[:], d["wo"][l].rearrange(
                    "p (hh x) -> p hh x", hh=2))
                wsq_t = wp.tile([128, 3], F32, tag="wsq")
                nc.sync.dma_start(wsq_t[:], d["wsq"][l])
                ngq_t = wp.tile([128, 3], F32, tag="ngq")
                nc.sync.dma_start(ngq_t[:], d["ngq"][l])
                bo8_t = wp.tile([128, 8], F32, tag="bo8")
                nc.sync.dma_start(bo8_t[:], d["bo8"][l])

                for b in range(2):
                    if l > 0:
                        resid_add(b)        # previous layer's mlp AR
                    attn_sublayer(l, b, wqkv_t, wo_t, wsq_t, ngq_t, bo8_t)

                w1_t = wp.tile([128, 8, 512], BF16, tag="w1")
                for q in range(2):
                    nc.sync.dma_start(
                        w1_t[:, q * 4:(q + 1) * 4, :],
                        d["w1"][l].rearrange("p (kc x) -> p kc x", kc=8)
                        [:, q * 4:(q + 1) * 4, :])
                w2_t = wp.tile([128, 4, 1024], BF16, tag="w2")
                for q in range(2):
                    nc.sync.dma_start(
                        w2_t[:, q * 2:(q + 1) * 2, :],
                        d["w2"][l].rearrange("p (kc x) -> p kc x", kc=4)
                        [:, q * 2:(q + 1) * 2, :])
                ws1_t = wp.tile([128, 4], F32, tag="ws1")
                nc.sync.dma_start(ws1_t[:], d["ws1"][l])
                ng1_t = wp.tile([128, 4], F32, tag="ng1")
                nc.sync.dma_start(ng1_t[:], d["ng1"][l])
                b28_t = wp.tile([128, 8], F32, tag="b28")
                nc.sync.dma_start(b28_t[:], d["b28"][l])

                for b in range(2):
                    resid_add(b)            # attn AR
                    mlp_sublayer(l, b, w1_t, w2_t, ws1_t, ng1_t, b28_t)

            # ---------------- tail: final norm, CA, logits ----------------
            embT_t = sbp.tile([128, 8, 256], BF16, tag="embT")
            nc.sync.dma_start(embT_t[:], d["embT"][:].rearrange(
                "p (kc x) -> p kc x", kc=8))
            headw_t = sbp.tile([128, 8, 256], BF16, tag="headw")
            nc.sync.dma_start(headw_t[:], d["headw"][:].rearrange(
                "p (kc x) -> p kc x", kc=8))
            wq_t = cwp.tile([128, 8, 128], BF16, tag="wqca")
            nc.sync.dma_start(wq_t[:], d["wq"][:].rearrange(
                "p (kc x) -> p kc x", kc=8))
            wk_t = cwp.tile([128, 8, 128], BF16, tag="wkca")
            nc.sync.dma_start(wk_t[:], d["wk"][:].rearrange(
                "p (kc x) -> p kc x", kc=8))
            wv_t = cwp.tile([128, 8, 128], BF16, tag="wvca")
            nc.sync.dma_start(wv_t[:], d["wv"][:].rearrange(
                "p (kc x) -> p kc x", kc=8))
            cawoT_t = cwp.tile([128, 8, 128], BF16, tag="cawoT")
            nc.sync.dma_start(cawoT_t[:], d["cawoT"][:].rearrange(
                "p (kc x) -> p kc x", kc=8))

            # qn = ln(embT)*cag + cab (shared between batches)
            qn_t = sbp.tile([128, 8, 256], BF16, tag="qn")
            rsb, msb = stats(embT_t, "bf16", 0)
            for ti in range(8):
                t1 = tp.tile([128, 256], F32R, tag="fx1")
                nc.vector.tensor_tensor(out=t1[:], in0=embT_t[:, ti, :],
                                        in1=rsb[:], op=ALU.mult)
                t2 = tp.tile([128, 256], F32R, tag="fx2")
                nc.vector.tensor_tensor(out=t2[:], in0=t1[:], in1=msb[:],
                                        op=ALU.subtract)
                nc.vector.tensor_scalar(out=qn_t[:, ti, :], in0=t2[:],
                                        scalar1=cag_t[:, ti:ti + 1],
                                        scalar2=cab_t[:, ti:ti + 1],
                                        op0=ALU.mult, op1=ALU.add)

            # qT (shared vocab queries)
            qT_t = sbp.tile([128, 256], BF16, tag="qT")
            ps = pp.tile([128, 256], F32, tag="mm")
            for kc in range(8):
                nc.tensor.matmul(ps[:], wq_t[:, kc, :], qn_t[:, kc, :],
                                 start=(kc == 0), stop=(kc == 7))
            nc.vector.tensor_scalar(out=qT_t[:], in0=ps[:],
                                    scalar1=bq_t[:], scalar2=None,
                                    op0=ALU.add)

            # w2c = cawoT.T @ headw (shared) and emb@head_w term (shared)
            w2c_t = sbp.tile([128, 256], BF16, tag="w2c")
            ps = pp.tile([128, 256], F32, tag="mm")
            for kc in range(8):
                nc.tensor.matmul(ps[:], cawoT_t[:, kc, :], headw_t[:, kc, :],
                                 start=(kc == 0), stop=(kc == 7))
            nc.vector.tensor_copy(w2c_t[:], ps[:])
            et_t = sbp.tile([128, 2, 256], F32, tag="et")
            for lt in range(2):
                ps_e = pp.tile([128, 256], F32, tag="mm")
                for kc in range(8):
                    nc.tensor.matmul(ps_e[:],
                                     headw_t[:, kc, lt * 128:(lt + 1) * 128],
                                     embT_t[:, kc, :],
                                     start=(kc == 0), stop=(kc == 7))
                nc.vector.tensor_copy(et_t[:, lt, :], ps_e[:])

            # per-batch: final norm -> kvn, CA, logits partial, AR
            kvn_t = [None, None]
            lar_t = [sbp.tile([128, 2, 256], FP16, name=f"lar{b}", tag=f"lar{b}")
                     for b in range(2)]
            for b in range(2):
                resid_add(b)                # last mlp AR
                rsb, msb = stats(h_t[b], "f32r", b)
                for ti in range(8):
                    t1 = tp.tile([128, 256], F32R, tag="fx1")
                    nc.vector.tensor_tensor(out=t1[:], in0=h_t[b][:, ti, :],
                                            in1=rsb[:], op=ALU.mult)
                    t2 = tp.tile([128, 256], F32R, tag="fx2")
                    nc.vector.tensor_tensor(out=t2[:], in0=t1[:], in1=msb[:],
                                            op=ALU.subtract)
                    nc.vector.tensor_scalar(out=h16_t[b][:, ti, :], in0=t2[:],
                                            scalar1=fng_t[:, ti:ti + 1],
                                            scalar2=fnb_t[:, ti:ti + 1],
                                            op0=ALU.mult, op1=ALU.add)
                if skip_kvn_ln:
                    kvn_t[b] = h16_t[b]
                else:
                    kvn_t[b] = sbp.tile([128, 8, 256], BF16, tag=f"kvn{b}")
                    rsb, msb = stats(h16_t[b], "bf16", b)
                    for ti in range(8):
                        t1 = tp.tile([128, 256], F32R, tag="fx1")
                        nc.vector.tensor_tensor(out=t1[:],
                                                in0=h16_t[b][:, ti, :],
                                                in1=rsb[:], op=ALU.mult)
                        t2 = tp.tile([128, 256], F32R, tag="fx2")
                        nc.vector.tensor_tensor(out=t2[:], in0=t1[:],
                                                in1=msb[:], op=ALU.subtract)
                        nc.vector.tensor_scalar(out=kvn_t[b][:, ti, :],
                                                in0=t2[:],
                                                scalar1=cag_t[:, ti:ti + 1],
                                                scalar2=cab_t[:, ti:ti + 1],
                                                op0=ALU.mult, op1=ALU.add)

                kT_t = sbp.tile([128, 256], BF16, tag=f"kT{b}")
                vT_t = sbp.tile([128, 256], BF16, tag=f"vT{b}")
                for (w_v, bias_t, out_t) in ((wk_t, bk_t, kT_t),
                                             (wv_t, bv_t, vT_t)):
                    ps = pp.tile([128, 256], F32, tag="mm")
                    for kc in range(8):
                        nc.tensor.matmul(ps[:], w_v[:, kc, :],
                                         kvn_t[b][:, kc, :],
                                         start=(kc == 0), stop=(kc == 7))
                    nc.vector.tensor_scalar(out=out_t[:], in0=ps[:],
                                            scalar1=bias_t[:], scalar2=None,
                                            op0=ALU.add)

                em = tp.tile([128, 2, 256], BF16, tag="em")
                for kt in range(2):
                    ps_s = pa.tile([128, 256], F32, tag="att")
                    nc.tensor.matmul(
                        ps_s[:], kT_t[:, kt * 128:(kt + 1) * 128],
                        qT_t[:], start=True, stop=True)
                    nc.scalar.activation(em[:, kt, :], ps_s[:], AF.Exp,
                                         scale=float(1.0 / np.sqrt(128.0)))
                ps_d = pst.tile([1, 256], F32, tag="stat")
                for kt in range(2):
                    nc.tensor.matmul(ps_d[:], ones16_t[:, 0:1], em[:, kt, :],
                                     start=(kt == 0), stop=(kt == 1))
                rec = tps.tile([1, 256], F32, tag="rec")
                nc.vector.reciprocal_approx_fast(out=rec[:], in_=ps_d[:])
                ps_rb = pp.tile([128, 256], F32, tag="mm")
                nc.tensor.matmul(ps_rb[:], onesf_t[0:1, :], rec[:],
                                 start=True, stop=True)
                rec_b = tp.tile([128, 256], F32R, tag="recb")
                nc.vector.tensor_copy(rec_b[:], ps_rb[:])
                vtok = tp.tile([128, 2, 128], BF16, tag="vtokca")
                for kt in range(2):
                    ps_t = pa.tile([128, 256], BF16, tag="att")
                    nc.tensor.transpose(
                        ps_t[:, :128],
                        vT_t[:, kt * 128:(kt + 1) * 128],
                        ident_t[:])
                    nc.vector.tensor_copy(vtok[:, kt, :], ps_t[:, :128])
                ps_o = pa.tile([128, 256], F32, tag="att")
                for kt in range(2):
                    nc.tensor.matmul(ps_o[:], vtok[:, kt, :], em[:, kt, :],
                                     start=(kt == 0), stop=(kt == 1))
                O_t = tp.tile([128, 256], BF16, tag="O")
                nc.vector.tensor_tensor(out=O_t[:], in0=ps_o[:],
                                        in1=rec_b[:], op=ALU.mult)

                lp_t = sbp.tile([128, 2, 256], FP16, tag=f"lp{b}")
                for lt in range(2):
                    ps = pp.tile([128, 256], F32, tag="mm")
                    nc.tensor.matmul(ps[:],
                                     w2c_t[:, lt * 128:(lt + 1) * 128],
                                     O_t[:], start=True, stop=True)
                    nc.vector.tensor_copy(lp_t[:, lt, :], ps[:])
                lbin = dp.tile([128, 512], FP16, tag=f"lci{b}")
                lbout = dp.tile([128, 512], FP16, addr_space="Shared",
                                tag=f"lco{b}")
                nc.sync.dma_start(lbin[:], lp_t[:])
                nc.gpsimd.collective_compute(
                    "AllReduce", ALU.add, replica_groups=RG,
                    ins=[lbin[:].opt()], outs=[lbout[:].opt()])
                nc.sync.dma_start(lar_t[b][:], lbout[:])

            for b in range(2):
                out_t = sbp.tile([128, 2, 256], F32, tag=f"outt{b}")
                for lt in range(2):
                    tb = tp.tile([128, 256], F32, tag="tb")
                    nc.vector.tensor_scalar(out=tb[:], in0=lar_t[b][:, lt, :],
                                            scalar1=headb_t[:, lt:lt + 1],
                                            scalar2=None, op0=ALU.add)
                    nc.vector.tensor_tensor(out=out_t[:, lt, :],
                                            in0=tb[:], in1=et_t[:, lt, :],
                                            op=ALU.add)
                nc.sync.dma_start(ltab_v[:, :, b, :], out_t[:])

    nc.compile()
    nc.m = get_hw_module(nc.m)
    return nc


# --------------------------------------------------------------------------
# host side
# --------------------------------------------------------------------------
def _shuf(M):
    """[K, X] -> [128, (K//128)*X] laid out as [p, kc, x]."""
    K, X = M.shape
    return np.ascontiguousarray(
        M.reshape(K // 128, 128, X).transpose(1, 0, 2).reshape(128, -1))


def _bf(M):
    return np.ascontiguousarray(M).astype(BF)


def _prep(inputs):
    f = lambda k: np.asarray(inputs[k], np.float32)
    byte_seq = np.asarray(inputs["byte_seq"])
    bd = np.asarray(inputs["patch_boundaries"])
    emb = f("emb")

    # patch histogram matrix
    pos = np.arange(S)
    pid = np.stack([np.searchsorted(bd[b], pos, side="right") for b in range(B)])
    pid = np.clip(pid, 0, P - 1)
    Cn = np.zeros((B, P, V), np.float32)
    for b in range(B):
        np.add.at(Cn[b], (pid[b], byte_seq[b]), 1.0)
    cnts = Cn.sum(-1)
    Cn /= np.maximum(cnts, 1.0)[..., None]
    cnt_all = np.concatenate([Cn[0].T, Cn[1].T], axis=1)  # [V, 512]

    g1, b1a = f("g_ln1_g"), f("g_ln1_b")
    g2, b2a = f("g_ln2_g"), f("g_ln2_b")
    Wqkv, bqkv = f("g_wqkv"), f("g_bqkv")
    Wo, bo = f("g_wo"), f("g_bo")
    W1, b1 = f("g_w1"), f("g_b1")
    W2, b2 = f("g_w2"), f("g_b2")

    Wq_f = g1[:, :, None] * Wqkv                       # [L, H, 3H]
    biasq = np.einsum("lh,lho->lo", b1a, Wqkv) + bqkv  # [L, 3H]
    wsumq = Wq_f.sum(1)                                # [L, 3H]
    W1_f = g2[:, :, None] * W1
    bias1 = np.einsum("lh,lho->lo", b2a, W1) + b1
    wsum1 = W1_f.sum(1)

    ca_wqkv, ca_bqkv = f("ca_wqkv"), f("ca_bqkv")
    ca_wo, ca_bo = f("ca_wo"), f("ca_bo")
    head_w, head_b = f("head_w"), f("head_b")
    headb_full = head_b + ca_bo @ head_w               # [256]

    masks = np.zeros((128, 2, 256), np.float32)
    for kt in range(2):
        ktg = kt * 128 + np.arange(128)
        masks[:, kt, :] = (ktg[:, None] <= np.arange(256)[None, :])

    shared = {
        "headw": _bf(_shuf(head_w)),
        "headb": np.ascontiguousarray(headb_full.reshape(2, 128).T),
        "embT": _bf(_shuf(np.ascontiguousarray(emb.T))),
        "embS": _bf(_shuf(emb)),
        "cnt": _bf(_shuf(cnt_all)),
        "masks": _bf(masks.reshape(128, 512)),
        "ones16": _bf(np.ones((128, 128), np.float32)),
        "onesr": np.ones((128, 128), np.float32),
        "ident": _bf(np.eye(128, dtype=np.float32)),
        "fng": np.ascontiguousarray(f("fn_g").reshape(8, 128).T),
        "fnb": np.ascontiguousarray(f("fn_b").reshape(8, 128).T),
        "cag": np.ascontiguousarray(f("ca_ln_g").reshape(8, 128).T),
        "cab": np.ascontiguousarray(f("ca_ln_b").reshape(8, 128).T),
        "bo8": np.ascontiguousarray(
            bo.reshape(L, 8, 128).transpose(0, 2, 1) / NC),
        "b28": np.ascontiguousarray(
            b2.reshape(L, 8, 128).transpose(0, 2, 1) / NC),
    }

    in_maps = []
    for c in range(NC):
        cols = np.concatenate([np.arange(c * 128, (c + 1) * 128) + k * H
                               for k in range(3)])
        m = dict(shared)
        m["wqkv"] = _bf(np.stack([_shuf(Wq_f[l][:, cols]) for l in range(L)]))
        m["wsq"] = np.ascontiguousarray(
            wsumq[:, cols].reshape(L, 3, 128).transpose(0, 2, 1))
        m["ngq"] = np.ascontiguousarray(
            (-biasq[:, cols]).reshape(L, 3, 128).transpose(0, 2, 1))
        m["wo"] = _bf(Wo[:, c * 128:(c + 1) * 128, :]
                      .reshape(L, 2, 64, H).transpose(0, 2, 1, 3)
                      .reshape(L, 64, 2 * H))
        m["w1"] = _bf(np.stack([_shuf(W1_f[l][:, c * 512:(c + 1) * 512])
                                for l in range(L)]))
        m["ws1"] = np.ascontiguousarray(
            wsum1[:, c * 512:(c + 1) * 512].reshape(L, 4, 128)
            .transpose(0, 2, 1))
        m["ng1"] = np.ascontiguousarray(
            (-bias1[:, c * 512:(c + 1) * 512]).reshape(L, 4, 128)
            .transpose(0, 2, 1))
        m["w2"] = _bf(np.stack([_shuf(W2[l][c * 512:(c + 1) * 512, :])
                                for l in range(L)]))
        m["wq"] = _bf(_shuf(ca_wqkv[:, c * 128:(c + 1) * 128]))
        m["wk"] = _bf(_shuf(ca_wqkv[:, H + c * 128: H + (c + 1) * 128]))
        m["wv"] = _bf(_shuf(ca_wqkv[:, 2 * H + c * 128: 2 * H + (c + 1) * 128]))
        m["bq"] = np.ascontiguousarray(
            ca_bqkv[c * 128:(c + 1) * 128, None])
        m["bk"] = np.ascontiguousarray(
            ca_bqkv[H + c * 128: H + (c + 1) * 128, None])
        m["bv"] = np.ascontiguousarray(
            ca_bqkv[2 * H + c * 128: 2 * H + (c + 1) * 128, None])
        m["cawoT"] = _bf(_shuf(np.ascontiguousarray(
            ca_wo[c * 128:(c + 1) * 128, :].T)))
        in_maps.append(m)
    return in_maps, byte_seq


def run_device(inputs, trace=False):
    skip = (np.allclose(np.asarray(inputs["fn_g"]), 1.0)
            and np.allclose(np.asarray(inputs["fn_b"]), 0.0)
            and np.allclose(np.asarray(inputs["ca_ln_g"]), 1.0)
            and np.allclose(np.asarray(inputs["ca_ln_b"]), 0.0))
    key = ("nc", skip)
    if key not in _CACHE:
        _CACHE[key] = _trace(skip)
    nc = _CACHE[key]
    in_maps, byte_seq = _prep(inputs)
    res = run_bass_kernel_spmd(nc, in_maps, core_ids=list(range(NC)),
                               trace=trace)
    ltab = res.results[0]["ltab"]                     # [128, 1024]
    ltab = ltab.reshape(128, 2, 2, 256)
    out = np.empty((B, S, V), np.float32)
    for b in range(B):
        tab = ltab[:, :, b, :].transpose(1, 0, 2).reshape(256, 256)
        out[b] = tab.T[byte_seq[b]]                   # [S, 256]
    return out, res


def kernel(**inputs) -> np.ndarray:
    out, _ = run_device(inputs, trace=False)
    return out


# revision 14
# speedup vs baseline: 1.0400x; 1.0344x over previous
"""BLT model TRN2 kernel — nn_BLTModel_13872744366807.

Strategy v4:
- Vocab collapse: the byte-axis path depends only on byte VALUE (V=256),
  so the [B,4096,*] byte axis collapses to a [B,256,*] table; patch
  mean-pooling becomes a host-computed histogram matrix times emb; final
  output is a host gather.
- DP-2 x TP-4: cores 0-3 run batch 0, cores 4-7 batch 1. Within each
  4-core group, Megatron TP over heads/hidden. Each batch's 256 patch
  tokens are split into 2 chunks of 128 that run as two pipelined
  streams: chunk 0 computes while chunk 1's fp16 AllReduce (256 KB,
  4-rank group) flies on the TOPSP/SDMA engines, and vice versa. The
  two groups' collectives are independent and run concurrently.
- Causal attention per chunk: chunk 0 queries see keys 0-127 (causal
  mask); chunk 1 queries see keys 0-127 unmasked + keys 128-255 causal.
- bf16 weights + activation mirrors for all big matmuls; fp32 residual
  stream + LN stats; LayerNorm commuted through weight matmuls with
  host-folded affines + colsum fixups (exact).
"""
import numpy as np
import ml_dtypes
import concourse.bacc as bacc
import concourse.bass as bass
import concourse.mybir as mybir
from concourse import tile
from concourse.bass_utils import run_bass_kernel_spmd
from concourse.bass_interp import get_hw_module

F32 = mybir.dt.float32
F32R = mybir.dt.float32r
BF16 = mybir.dt.bfloat16
FP16 = mybir.dt.float16
AF = mybir.ActivationFunctionType
ALU = mybir.AluOpType
BF = ml_dtypes.bfloat16

L, B, S, P, H, V, NC = 4, 2, 4096, 256, 1024, 256, 8
G = 4
EPS = 1e-6
RG = [[0, 1, 2, 3], [4, 5, 6, 7]]

_CACHE = {}


# --------------------------------------------------------------------------
# device program
# --------------------------------------------------------------------------
def _trace(skip_kvn_ln):
    nc = bacc.Bacc("TRN2", target_bir_lowering=False, debug=False,
                   num_devices=NC)
    d = {}

    def inp(name, shape, dt=BF16):
        d[name] = nc.dram_tensor(name, shape, dt, kind="ExternalInput").ap()

    inp("wqkv", [L, 128, 6144])
    inp("wsq", [L, 128, 6], F32)
    inp("ngq", [L, 128, 6], F32)
    inp("wo", [L, 64, 4096])
    inp("bo8", [L, 128, 8], F32)
    inp("w1", [L, 128, 8192])
    inp("ws1", [L, 128, 8], F32)
    inp("ng1", [L, 128, 8], F32)
    inp("w2", [L, 128, 8192])
    inp("b28", [L, 128, 8], F32)
    inp("wq", [128, 2048]); inp("wk", [128, 2048]); inp("wv", [128, 2048])
    inp("bq", [128, 2], F32); inp("bk", [128, 2], F32); inp("bv", [128, 2], F32)
    inp("cawoT", [128, 2048])
    inp("headw", [128, 2048])
    inp("headb", [128, 2], F32)
    inp("embT", [128, 2048])
    inp("embS", [128, 2048])
    inp("cnt", [128, 512])
    inp("masks", [128, 128])
    inp("ones16", [128, 128])
    inp("onesr", [128, 128], F32R)
    inp("ident", [128, 128])
    inp("fng", [128, 8], F32); inp("fnb", [128, 8], F32)
    inp("cag", [128, 8], F32); inp("cab", [128, 8], F32)
    out_d = nc.dram_tensor("ltab", [128, 512], F32, kind="ExternalOutput").ap()

    with tile.TileContext(nc) as tc:
        with (
            tc.tile_pool(name="const", bufs=1) as cp,
            tc.tile_pool(name="sb", bufs=1) as sbp,
            tc.tile_pool(name="wts", bufs=2) as wp,
            tc.tile_pool(name="cwts", bufs=1) as cwp,
            tc.tile_pool(name="wts1", bufs=1) as wps,
            tc.tile_pool(name="tmp", bufs=3) as tp,
            tc.tile_pool(name="tps", bufs=2) as tps,
            tc.tile_pool(name="pp", bufs=3, space="PSUM") as pp,
            tc.tile_pool(name="pa", bufs=3, space="PSUM") as pa,
            tc.tile_pool(name="pst", bufs=2, space="PSUM") as pst,
            tc.tile_pool(name="dram", bufs=1, space="DRAM") as dp,
        ):
            # cc warm-up first: its DMA must not queue behind weight loads
            wbin = dp.tile([128, 8], F32, tag="wrmi")
            wbout = dp.tile([128, 8], F32, tag="wrmo")
            nc.sync.dma_start(wbin[:], d["bo8"][0].bitcast(F32))
            nc.gpsimd.collective_compute(
                "AllReduce", ALU.add, replica_groups=RG,
                ins=[wbin[:].opt()], outs=[wbout[:].opt()])

            # ---------------- constants ----------------
            def cload(name, shape, dt=BF16):
                t_ = cp.tile(shape, dt, tag=name)
                nc.sync.dma_start(t_[:], d[name][:])
                return t_

            embS_t = cp.tile([128, 2, 1024], BF16, tag="embS")
            nc.sync.dma_start(embS_t[:], d["embS"][:].rearrange(
                "p (vc x) -> p vc x", vc=2))
            cnt_t = cp.tile([128, 2, 2, 128], BF16, tag="cnt")
            nc.sync.dma_start(cnt_t[:], d["cnt"][:].rearrange(
                "p (vc c2 x) -> p vc c2 x", vc=2, c2=2))
            ones16_t = cload("ones16", [128, 128])
            onesr_t = cload("onesr", [128, 128], F32R)
            onesf_t = cp.tile([1, 128], F32, tag="onesf")
            nc.sync.dma_start(onesf_t[:], d["onesr"][0:1, :].bitcast(F32))
            ident_t = cload("ident", [128, 128])
            masks_t = cload("masks", [128, 128])
            fng_t = cload("fng", [128, 8], F32); fnb_t = cload("fnb", [128, 8], F32)
            cag_t = cload("cag", [128, 8], F32); cab_t = cload("cab", [128, 8], F32)
            headb_t = cload("headb", [128, 2], F32)
            bq_t = cload("bq", [128, 2], F32); bk_t = cload("bk", [128, 2], F32)
            bv_t = cload("bv", [128, 2], F32)

            # ---------------- persistent activations (per chunk) -----------
            def per_c2(shape, dt, nm):
                return [sbp.tile(shape, dt, name=f"{nm}{c}", tag=f"{nm}{c}")
                        for c in range(2)]

            h_t = per_c2([128, 8, 128], F32R, "h")
            h16_t = per_c2([128, 8, 128], BF16, "h16_")
            sq_t = per_c2([128, 8, 128], F32R, "sq")
            qkv_t = per_c2([128, 2, 3, 128], BF16, "qkv")
            qkvh2_t = per_c2([64, 2, 3, 128], BF16, "qkvh2_")
            A_t = per_c2([64, 2, 2, 128], BF16, "A")
            gu_t = per_c2([128, 8, 128], BF16, "gu")
            aro_t = per_c2([128, 8, 128], FP16, "aro")
            ari_t = per_c2([128, 8, 128], FP16, "ari")

            # ---------------- helpers ----------------
            def stats(src, srcdt, c2, w=128):
                ones_src = onesr_t if srcdt == "f32r" else ones16_t
                sq = sq_t[c2]
                ps_sum = pst.tile([1, 256], F32, tag="stat")
                ps_sq = pst.tile([1, 256], F32, tag="stat")
                for ti in range(8):
                    nc.scalar.activation(sq[:, ti, :w], src[:, ti, :],
                                         AF.Square)
                for ti in range(8):
                    nc.tensor.matmul(ps_sum[:, :w], ones_src[:, 0:1],
                                     src[:, ti, :],
                                     start=(ti == 0), stop=(ti == 7))
                for ti in range(8):
                    nc.tensor.matmul(ps_sq[:, :w], onesr_t[:, 0:1],
                                     sq[:, ti, :w],
                                     start=(ti == 0), stop=(ti == 7))
                inv = 1.0 / 1024.0
                mu = tps.tile([1, 128], F32R, tag="mu")
                nc.vector.tensor_scalar_mul(mu[:], ps_sum[:, :w], inv)
                ex2 = tps.tile([1, 128], F32R, tag="ex2")
                nc.vector.tensor_scalar(out=ex2[:], in0=ps_sq[:, :w],
                                        scalar1=inv, scalar2=EPS,
                                        op0=ALU.mult, op1=ALU.add)
                mus = tps.tile([1, 128], F32R, tag="mus")
                nc.scalar.activation(mus[:], mu[:], AF.Square)
                vare = tps.tile([1, 128], F32, tag="var")
                nc.vector.tensor_tensor(out=vare[:], in0=ex2[:],
                                        in1=mus[:], op=ALU.subtract)
                vrec = tps.tile([1, 128], F32, tag="vrec")
                nc.vector.reciprocal_approx_fast(out=vrec[:], in_=vare[:])
                rsig = tps.tile([1, 128], F32R, tag="rsig")
                nc.scalar.activation(rsig[:], vrec[:], AF.Sqrt)
                musg = tps.tile([1, 128], F32R, tag="musg")
                nc.vector.tensor_tensor(out=musg[:], in0=mu[:],
                                        in1=rsig[:], op=ALU.mult)
                pb = pp.tile([128, 256], F32, tag="mm")
                nc.tensor.matmul(pb[:, :w], onesr_t[0:1, :], rsig[:],
                                 start=True, stop=True)
                rsig_b = tp.tile([128, 128], F32R, tag="rsigb")
                nc.vector.tensor_copy(rsig_b[:], pb[:, :w])
                pb2 = pp.tile([128, 256], F32, tag="mm")
                nc.tensor.matmul(pb2[:, :w], onesr_t[0:1, :], musg[:],
                                 start=True, stop=True)
                musig_b = tp.tile([128, 128], F32R, tag="musgb")
                nc.vector.tensor_copy(musig_b[:], pb2[:, :w])
                return rsig_b, musig_b

            def fixup(ps, mcol, rsig_b, musig_b, wsum_t, negb_t, out_ap,
                      gelu=False):
                t1 = tp.tile([128, 128], F32R, tag="fx1")
                nc.vector.tensor_tensor(out=t1[:], in0=ps[:], in1=rsig_b[:],
                                        op=ALU.mult)
                m2 = tp.tile([128, 128], F32R, tag="fx2")
                nc.vector.tensor_scalar(out=m2[:], in0=musig_b[:],
                                        scalar1=wsum_t[:, mcol:mcol + 1],
                                        scalar2=negb_t[:, mcol:mcol + 1],
                                        op0=ALU.mult, op1=ALU.add)
                if gelu:
                    t2 = tp.tile([128, 128], F32R, tag="fx3")
                    nc.vector.tensor_tensor(out=t2[:], in0=t1[:], in1=m2[:],
                                            op=ALU.subtract)
                    nc.scalar.activation(out_ap, t2[:], AF.Gelu)
                else:
                    nc.vector.tensor_tensor(out=out_ap, in0=t1[:], in1=m2[:],
                                            op=ALU.subtract)

            def allreduce_fp16(tag, c2):
                bin_ = dp.tile([128, 1024], FP16, tag=f"ci{tag}")
                bout = dp.tile([128, 1024], FP16, tag=f"co{tag}")
                for q in range(8):
                    nc.sync.dma_start(bin_[:, q * 128:(q + 1) * 128],
                                      aro_t[c2][:, q, :])
                nc.gpsimd.collective_compute(
                    "AllReduce", ALU.add, replica_groups=RG,
                    ins=[bin_[:].opt()], outs=[bout[:].opt()])
                for q in range(8):
                    nc.sync.dma_start(ari_t[c2][:, q, :],
                                      bout[:, q * 128:(q + 1) * 128])

            def resid_add(c2):
                for ti in range(8):
                    nc.vector.tensor_tensor(out=h_t[c2][:, ti, :],
                                            in0=h_t[c2][:, ti, :],
                                            in1=ari_t[c2][:, ti, :],
                                            op=ALU.add)
                    nc.scalar.activation(h16_t[c2][:, ti, :],
                                         h_t[c2][:, ti, :], AF.Copy)

            # ---------------- patch pooling ----------------
            for c2 in range(2):
                for ti in range(8):
                    ps = pp.tile([128, 256], F32, tag="mm")
                    for vc in range(2):
                        nc.tensor.matmul(
                            ps[:, :128],
                            embS_t[:, vc, ti * 128:(ti + 1) * 128],
                            cnt_t[:, vc, c2, :],
                            start=(vc == 0), stop=(vc == 1))
                    nc.vector.tensor_copy(h_t[c2][:, ti, :], ps[:, :128])
                    nc.scalar.activation(h16_t[c2][:, ti, :], ps[:, :128],
                                         AF.Copy)

            # ---------------- transformer layers ----------------
            def attn_sublayer(l, c2, wqkv_t, wo_t, wsq_t, ngq_t, bo8_t):
                rsb, msb = stats(h_t[c2], "f32r", c2)
                for p in range(2):
                    for j in range(3):
                        m = p * 3 + j
                        ps = pp.tile([128, 256], F32, tag="mm")
                        for kc in range(8):
                            nc.tensor.matmul(
                                ps[:, :128],
                                wqkv_t[:, kc, m * 128:(m + 1) * 128],
                                h16_t[c2][:, kc, :],
                                start=(kc == 0), stop=(kc == 7))
                        fixup(ps[:, :128], m, rsb, msb, wsq_t, ngq_t,
                              qkv_t[c2][:, p, j, :])
                nc.sync.dma_start(qkvh2_t[c2][:], qkv_t[c2][64:128, :, :, :])

                # attention: queries = this chunk; keys/values = chunks<=c2
                for p in range(2):
                    for hh in range(2):
                        src = [qkv_t[0][:, p], qkv_t[1][:, p]] if hh == 0 \
                            else [qkvh2_t[0][:, p], qkvh2_t[1][:, p]]
                        qT = src[c2][0:64, 0, :]
                        nkt = c2 + 1
                        vtok = tp.tile([128, 2, 64], BF16, tag="vtok")
                        for kt in range(nkt):
                            ps_t = pa.tile([128, 256], BF16, tag="att")
                            nc.tensor.transpose(ps_t[:, :64],
                                                src[kt][0:64, 2, :],
                                                ident_t[0:64, 0:64])
                            nc.vector.tensor_copy(vtok[:, kt, :],
                                                  ps_t[:, :64])
                        em = tp.tile([128, 2, 128], BF16, tag="em")
                        for kt in range(nkt):
                            ps_s = pa.tile([128, 256], F32, tag="att")
                            nc.tensor.matmul(ps_s[:, :128],
                                             src[kt][0:64, 1, :],
                                             qT[:], start=True, stop=True)
                            if kt == c2:   # causal block: mask needed
                                ex = tp.tile([128, 128], BF16, tag="ex")
                                nc.scalar.activation(ex[:], ps_s[:, :128],
                                                     AF.Exp, scale=0.125)
                                nc.vector.tensor_tensor(
                                    out=em[:, kt, :], in0=ex[:],
                                    in1=masks_t[:], op=ALU.mult)
                            else:          # fully visible block
                                nc.scalar.activation(em[:, kt, :],
                                                     ps_s[:, :128],
                                                     AF.Exp, scale=0.125)
                        ps_d = pst.tile([1, 256], F32, tag="stat")
                        for kt in range(nkt):
                            nc.tensor.matmul(ps_d[:, :128], ones16_t[:, 0:1],
                                             em[:, kt, :],
                                             start=(kt == 0),
                                             stop=(kt == nkt - 1))
                        rec = tps.tile([1, 128], F32, tag="rec")
                        nc.vector.reciprocal_approx_fast(out=rec[:],
                                                         in_=ps_d[:, :128])
                        ps_rb = pp.tile([128, 256], F32, tag="mm")
                        nc.tensor.matmul(ps_rb[:, :128], onesf_t[0:1, :],
                                         rec[:], start=True, stop=True)
                        rec_b = tp.tile([128, 128], F32R, tag="recb")
                        nc.vector.tensor_copy(rec_b[:], ps_rb[:, :128])
                        ps_o = pa.tile([128, 256], F32, tag="att")
                        for kt in range(nkt):
                            nc.tensor.matmul(
                                ps_o[0:64, :128], vtok[:, kt, :],
                                em[:, kt, :], start=(kt == 0),
                                stop=(kt == nkt - 1))
                        nc.vector.tensor_tensor(
                            out=A_t[c2][:, p, hh, :], in0=ps_o[0:64, :128],
                            in1=rec_b[0:64, :], op=ALU.mult)

                for m in range(8):
                    ps = pp.tile([128, 256], F32, tag="mm")
                    for p in range(2):
                        for hh in range(2):
                            nc.tensor.matmul(
                                ps[:, :128],
                                wo_t[0:64, p * 2 + hh,
                                     m * 128:(m + 1) * 128],
                                A_t[c2][:, p, hh, :],
                                start=(p == 0 and hh == 0),
                                stop=(p == 1 and hh == 1))
                    nc.vector.tensor_scalar(out=aro_t[c2][:, m, :],
                                            in0=ps[:, :128],
                                            scalar1=bo8_t[:, m:m + 1],
                                            scalar2=None, op0=ALU.add)
                allreduce_fp16(f"a{l}c{c2}", c2)

            def mlp_sublayer(l, c2, w1_t, w2_t, ws1_t, ng1_t, b28_t):
                rsb, msb = stats(h_t[c2], "f32r", c2)
                for m in range(8):
                    ps = pp.tile([128, 256], F32, tag="mm")
                    for kc in range(8):
                        nc.tensor.matmul(
                            ps[:, :128], w1_t[:, kc, m * 128:(m + 1) * 128],
                            h16_t[c2][:, kc, :],
                            start=(kc == 0), stop=(kc == 7))
                    fixup(ps[:, :128], m, rsb, msb, ws1_t, ng1_t,
                          gu_t[c2][:, m, :], gelu=True)
                for m in range(8):
                    ps = pp.tile([128, 256], F32, tag="mm")
                    for kc in range(8):
                        nc.tensor.matmul(
                            ps[:, :128], w2_t[:, kc, m * 128:(m + 1) * 128],
                            gu_t[c2][:, kc, :],
                            start=(kc == 0), stop=(kc == 7))
                    nc.vector.tensor_scalar(out=aro_t[c2][:, m, :],
                                            in0=ps[:, :128],
                                            scalar1=b28_t[:, m:m + 1],
                                            scalar2=None, op0=ALU.add)
                allreduce_fp16(f"m{l}c{c2}", c2)

            for l in range(4):
                wqkv_t = wp.tile([128, 8, 768], BF16, tag="wqkv")
                for q in range(2):
                    nc.sync.dma_start(
                        wqkv_t[:, q * 4:(q + 1) * 4, :],
                        d["wqkv"][l].rearrange("p (kc x) -> p kc x", kc=8)
                        [:, q * 4:(q + 1) * 4, :])
                wo_t = wp.tile([64, 4, 1024], BF16, tag="wo")
                nc.sync.dma_start(wo_t[:], d["wo"][l].rearrange(
                    "p (hh x) -> p hh x", hh=4))
                wsq_t = wp.tile([128, 6], F32, tag="wsq")
                nc.sync.dma_start(wsq_t[:], d["wsq"][l])
                ngq_t = wp.tile([128, 6], F32, tag="ngq")
                nc.sync.dma_start(ngq_t[:], d["ngq"][l])
                bo8_t = wp.tile([128, 8], F32, tag="bo8")
                nc.sync.dma_start(bo8_t[:], d["bo8"][l])

                for c2 in range(2):
                    if l > 0:
                        resid_add(c2)
                    attn_sublayer(l, c2, wqkv_t, wo_t, wsq_t, ngq_t, bo8_t)

                w1_t = wps.tile([128, 8, 1024], BF16, tag="w1")
                for q in range(2):
                    nc.sync.dma_start(
                        w1_t[:, q * 4:(q + 1) * 4, :],
                        d["w1"][l].rearrange("p (kc x) -> p kc x", kc=8)
                        [:, q * 4:(q + 1) * 4, :])
                w2_t = wps.tile([128, 8, 1024], BF16, tag="w2")
                for q in range(2):
                    nc.sync.dma_start(
                        w2_t[:, q * 4:(q + 1) * 4, :],
                        d["w2"][l].rearrange("p (kc x) -> p kc x", kc=8)
                        [:, q * 4:(q + 1) * 4, :])
                ws1_t = wp.tile([128, 8], F32, tag="ws1")
                nc.sync.dma_start(ws1_t[:], d["ws1"][l])
                ng1_t = wp.tile([128, 8], F32, tag="ng1")
                nc.sync.dma_start(ng1_t[:], d["ng1"][l])
                b28_t = wp.tile([128, 8], F32, tag="b28")
                nc.sync.dma_start(b28_t[:], d["b28"][l])

                for c2 in range(2):
                    resid_add(c2)
                    mlp_sublayer(l, c2, w1_t, w2_t, ws1_t, ng1_t, b28_t)

            # ---------------- tail ----------------
            embT_t = sbp.tile([128, 8, 256], BF16, tag="embT")
            nc.sync.dma_start(embT_t[:], d["embT"][:].rearrange(
                "p (kc x) -> p kc x", kc=8))
            headw_t = sbp.tile([128, 8, 256], BF16, tag="headw")
            nc.sync.dma_start(headw_t[:], d["headw"][:].rearrange(
                "p (kc x) -> p kc x", kc=8))
            wq_t = cwp.tile([128, 8, 256], BF16, tag="wqca")
            nc.sync.dma_start(wq_t[:], d["wq"][:].rearrange(
                "p (kc x) -> p kc x", kc=8))
            wk_t = cwp.tile([128, 8, 256], BF16, tag="wkca")
            nc.sync.dma_start(wk_t[:], d["wk"][:].rearrange(
                "p (kc x) -> p kc x", kc=8))
            wv_t = cwp.tile([128, 8, 256], BF16, tag="wvca")
            nc.sync.dma_start(wv_t[:], d["wv"][:].rearrange(
                "p (kc x) -> p kc x", kc=8))
            cawoT_t = cwp.tile([128, 8, 256], BF16, tag="cawoT")
            nc.sync.dma_start(cawoT_t[:], d["cawoT"][:].rearrange(
                "p (kc x) -> p kc x", kc=8))

            # final norm per chunk -> kvn [128, 8, 256] (both chunks)
            kvn_t = sbp.tile([128, 8, 256], BF16, tag="kvn")
            for c2 in range(2):
                resid_add(c2)
                rsb, msb = stats(h_t[c2], "f32r", c2)
                csl = slice(c2 * 128, (c2 + 1) * 128)
                for ti in range(8):
                    t1 = tp.tile([128, 128], F32R, tag="fx1")
                    nc.vector.tensor_tensor(out=t1[:], in0=h_t[c2][:, ti, :],
                                            in1=rsb[:], op=ALU.mult)
                    t2 = tp.tile([128, 128], F32R, tag="fx2")
                    nc.vector.tensor_tensor(out=t2[:], in0=t1[:], in1=msb[:],
                                            op=ALU.subtract)
                    nc.vector.tensor_scalar(out=kvn_t[:, ti, csl], in0=t2[:],
                                            scalar1=fng_t[:, ti:ti + 1],
                                            scalar2=fnb_t[:, ti:ti + 1],
                                            op0=ALU.mult, op1=ALU.add)
            if not skip_kvn_ln:
                kvn2_t = sbp.tile([128, 8, 256], BF16, tag="kvn2")
                for c2 in range(2):
                    csl = slice(c2 * 128, (c2 + 1) * 128)
                    rsb, msb = stats(kvn_t[:, :, csl], "bf16", c2)
                    for ti in range(8):
                        t1 = tp.tile([128, 128], F32R, tag="fx1")
                        nc.vector.tensor_tensor(out=t1[:],
                                                in0=kvn_t[:, ti, csl],
                                                in1=rsb[:], op=ALU.mult)
                        t2 = tp.tile([128, 128], F32R, tag="fx2")
                        nc.vector.tensor_tensor(out=t2[:], in0=t1[:],
                                                in1=msb[:], op=ALU.subtract)
                        nc.vector.tensor_scalar(out=kvn2_t[:, ti, csl],
                                                in0=t2[:],
                                                scalar1=cag_t[:, ti:ti + 1],
                                                scalar2=cab_t[:, ti:ti + 1],
                                                op0=ALU.mult, op1=ALU.add)
                kvn_t = kvn2_t

            # qn = ln(embT)*cag + cab over 256 vocab (2 chunk-stats)
            qn_t = sbp.tile([128, 8, 256], BF16, tag="qn")
            for c2 in range(2):
                csl = slice(c2 * 128, (c2 + 1) * 128)
                rsb, msb = stats(embT_t[:, :, csl], "bf16", c2)
                for ti in range(8):
                    t1 = tp.tile([128, 128], F32R, tag="fx1")
                    nc.vector.tensor_tensor(out=t1[:],
                                            in0=embT_t[:, ti, csl],
                                            in1=rsb[:], op=ALU.mult)
                    t2 = tp.tile([128, 128], F32R, tag="fx2")
                    nc.vector.tensor_tensor(out=t2[:], in0=t1[:], in1=msb[:],
                                            op=ALU.subtract)
                    nc.vector.tensor_scalar(out=qn_t[:, ti, csl], in0=t2[:],
                                            scalar1=cag_t[:, ti:ti + 1],
                                            scalar2=cab_t[:, ti:ti + 1],
                                            op0=ALU.mult, op1=ALU.add)

            # ---------------- CA projections (2 heads, dh=128) --------------
            kT_t = sbp.tile([128, 2, 256], BF16, tag="kT")
            vT_t = sbp.tile([128, 2, 256], BF16, tag="vT")
            qT_t = sbp.tile([128, 2, 256], BF16, tag="qT")
            for h2 in range(2):
                for (w_v, bias_t, out_t, src_t) in (
                    (wk_t, bk_t, kT_t, kvn_t),
                    (wv_t, bv_t, vT_t, kvn_t),
                    (wq_t, bq_t, qT_t, qn_t),
                ):
                    ps = pp.tile([128, 256], F32, tag="mm")
                    for kc in range(8):
                        nc.tensor.matmul(ps[:],
                                         w_v[:, kc, h2 * 128:(h2 + 1) * 128],
                                         src_t[:, kc, :],
                                         start=(kc == 0), stop=(kc == 7))
                    nc.vector.tensor_scalar(out=out_t[:, h2, :], in0=ps[:],
                                            scalar1=bias_t[:, h2:h2 + 1],
                                            scalar2=None, op0=ALU.add)

            # ---------------- CA attention ----------------
            O_t = sbp.tile([128, 2, 256], BF16, tag="O")
            for h2 in range(2):
                em = tp.tile([128, 2, 256], BF16, tag="emca")
                for kt in range(2):
                    ps_s = pa.tile([128, 256], F32, tag="att")
                    nc.tensor.matmul(
                        ps_s[:], kT_t[:, h2, kt * 128:(kt + 1) * 128],
                        qT_t[:, h2, :], start=True, stop=True)
                    nc.scalar.activation(em[:, kt, :], ps_s[:], AF.Exp,
                                         scale=float(1.0 / np.sqrt(128.0)))
                ps_d = pst.tile([1, 256], F32, tag="stat")
                for kt in range(2):
                    nc.tensor.matmul(ps_d[:], ones16_t[:, 0:1], em[:, kt, :],
                                     start=(kt == 0), stop=(kt == 1))
                rec = tps.tile([1, 256], F32, tag="recca")
                nc.vector.reciprocal_approx_fast(out=rec[:], in_=ps_d[:])
                ps_rb = pp.tile([128, 256], F32, tag="mm")
                nc.tensor.matmul(ps_rb[:], onesf_t[0:1, :], rec[:],
                                 start=True, stop=True)
                rec_b = tp.tile([128, 256], F32R, tag="recbca")
                nc.vector.tensor_copy(rec_b[:], ps_rb[:])
                vtok = tp.tile([128, 2, 128], BF16, tag="vtokca")
                for kt in range(2):
                    ps_t = pa.tile([128, 256], BF16, tag="att")
                    nc.tensor.transpose(
                        ps_t[:, :128],
                        vT_t[:, h2, kt * 128:(kt + 1) * 128],
                        ident_t[:])
                    nc.vector.tensor_copy(vtok[:, kt, :], ps_t[:, :128])
                ps_o = pa.tile([128, 256], F32, tag="att")
                for kt in range(2):
                    nc.tensor.matmul(ps_o[:], vtok[:, kt, :], em[:, kt, :],
                                     start=(kt == 0), stop=(kt == 1))
                nc.vector.tensor_tensor(out=O_t[:, h2, :],
                                        in0=ps_o[:], in1=rec_b[:],
                                        op=ALU.mult)

            # ---------------- logits partials + AR ----------------
            et_t = sbp.tile([128, 2, 256], F32, tag="et")
            for lt in range(2):
                ps_e = pp.tile([128, 256], F32, tag="mm")
                for kc in range(8):
                    nc.tensor.matmul(ps_e[:],
                                     headw_t[:, kc, lt * 128:(lt + 1) * 128],
                                     embT_t[:, kc, :],
                                     start=(kc == 0), stop=(kc == 7))
                nc.vector.tensor_copy(et_t[:, lt, :], ps_e[:])

            w2c_t = sbp.tile([128, 2, 256], BF16, tag="w2c")
            for h2 in range(2):
                ps = pp.tile([128, 256], F32, tag="mm")
                for kc in range(8):
                    nc.tensor.matmul(ps[:],
                                     cawoT_t[:, kc, h2 * 128:(h2 + 1) * 128],
                                     headw_t[:, kc, :],
                                     start=(kc == 0), stop=(kc == 7))
                nc.vector.tensor_copy(w2c_t[:, h2, :], ps[:])

            lp_t = sbp.tile([128, 2, 256], FP16, tag="lp")
            for lt in range(2):
                ps = pp.tile([128, 256], F32, tag="mm")
                for h2 in range(2):
                    nc.tensor.matmul(ps[:],
                                     w2c_t[:, h2, lt * 128:(lt + 1) * 128],
                                     O_t[:, h2, :],
                                     start=(h2 == 0), stop=(h2 == 1))
                nc.vector.tensor_copy(lp_t[:, lt, :], ps[:])
            lbin = dp.tile([128, 512], FP16, tag="lci")
            lbout = dp.tile([128, 512], FP16, tag="lco")
            nc.sync.dma_start(lbin[:], lp_t[:])
            nc.gpsimd.collective_compute(
                "AllReduce", ALU.add, replica_groups=RG,
                ins=[lbin[:].opt()], outs=[lbout[:].opt()])
            lar_t = sbp.tile([128, 2, 256], FP16, tag="lar")
            nc.sync.dma_start(lar_t[:], lbout[:])

            out_t = sbp.tile([128, 2, 256], F32, tag="outt")
            for lt in range(2):
                tb = tp.tile([128, 256], F32, tag="tb")
                nc.vector.tensor_scalar(out=tb[:], in0=lar_t[:, lt, :],
                                        scalar1=headb_t[:, lt:lt + 1],
                                        scalar2=None, op0=ALU.add)
                nc.vector.tensor_tensor(out=out_t[:, lt, :],
                                        in0=tb[:], in1=et_t[:, lt, :],
                                        op=ALU.add)
            nc.sync.dma_start(out_d[:], out_t[:])

    nc.compile()
    nc.m = get_hw_module(nc.m)
    return nc


# --------------------------------------------------------------------------
# host side
# --------------------------------------------------------------------------
def _shuf(M):
    """[K, X] -> [128, (K//128)*X] laid out as [p, kc, x]."""
    K, X = M.shape
    return np.ascontiguousarray(
        M.reshape(K // 128, 128, X).transpose(1, 0, 2).reshape(128, -1))


def _bf(M):
    return np.ascontiguousarray(M).astype(BF)


def _prep(inputs):
    f = lambda k: np.asarray(inputs[k], np.float32)
    byte_seq = np.asarray(inputs["byte_seq"])
    bd = np.asarray(inputs["patch_boundaries"])
    emb = f("emb")

    # patch histogram matrix
    pos = np.arange(S)
    pid = np.stack([np.searchsorted(bd[b], pos, side="right") for b in range(B)])
    pid = np.clip(pid, 0, P - 1)
    Cn = np.zeros((B, P, V), np.float32)
    for b in range(B):
        np.add.at(Cn[b], (pid[b], byte_seq[b]), 1.0)
    cnts = Cn.sum(-1)
    Cn /= np.maximum(cnts, 1.0)[..., None]

    g1, b1a = f("g_ln1_g"), f("g_ln1_b")
    g2, b2a = f("g_ln2_g"), f("g_ln2_b")
    Wqkv, bqkv = f("g_wqkv"), f("g_bqkv")
    Wo, bo = f("g_wo"), f("g_bo")
    W1, b1 = f("g_w1"), f("g_b1")
    W2, b2 = f("g_w2"), f("g_b2")

    Wq_f = g1[:, :, None] * Wqkv                       # [L, H, 3H]
    biasq = np.einsum("lh,lho->lo", b1a, Wqkv) + bqkv  # [L, 3H]
    wsumq = Wq_f.sum(1)                                # [L, 3H]
    W1_f = g2[:, :, None] * W1
    bias1 = np.einsum("lh,lho->lo", b2a, W1) + b1
    wsum1 = W1_f.sum(1)

    ca_wqkv, ca_bqkv = f("ca_wqkv"), f("ca_bqkv")
    ca_wo, ca_bo = f("ca_wo"), f("ca_bo")
    head_w, head_b = f("head_w"), f("head_b")
    headb_full = head_b + ca_bo @ head_w               # [256]

    # intra-chunk causal [128 keys, 128 queries] mask (key <= query)
    cmask = (np.arange(128)[:, None] <= np.arange(128)[None, :])

    shared = {
        "headw": _bf(_shuf(head_w)),
        "headb": np.ascontiguousarray(headb_full.reshape(2, 128).T),
        "embT": _bf(_shuf(np.ascontiguousarray(emb.T))),
        "embS": _bf(_shuf(emb)),
        "masks": _bf(cmask.astype(np.float32)),
        "ones16": _bf(np.ones((128, 128), np.float32)),
        "onesr": np.ones((128, 128), np.float32),
        "ident": _bf(np.eye(128, dtype=np.float32)),
        "fng": np.ascontiguousarray(f("fn_g").reshape(8, 128).T),
        "fnb": np.ascontiguousarray(f("fn_b").reshape(8, 128).T),
        "cag": np.ascontiguousarray(f("ca_ln_g").reshape(8, 128).T),
        "cab": np.ascontiguousarray(f("ca_ln_b").reshape(8, 128).T),
    }

    in_maps = []
    for c in range(NC):
        g, r = c // G, c % G
        m = dict(shared)
        m["cnt"] = _bf(_shuf(np.ascontiguousarray(Cn[g].T)))

        # qkv columns: m-tile (p, j) = component j of head-pair p
        cols = np.concatenate([
            j * H + 256 * r + 128 * p + np.arange(128)
            for p in range(2) for j in range(3)])
        m["wqkv"] = _bf(np.stack([_shuf(Wq_f[l][:, cols]) for l in range(L)]))
        m["wsq"] = np.ascontiguousarray(
            wsumq[:, cols].reshape(L, 6, 128).transpose(0, 2, 1))
        m["ngq"] = np.ascontiguousarray(
            (-biasq[:, cols]).reshape(L, 6, 128).transpose(0, 2, 1))
        # wo rows: slot (p*2+hh) holds head (4r+2p+hh)'s 64 features
        m["wo"] = _bf(Wo[:, 256 * r:256 * (r + 1), :]
                      .reshape(L, 4, 64, H).transpose(0, 2, 1, 3)
                      .reshape(L, 64, 4 * H))
        m["bo8"] = np.ascontiguousarray(
            bo.reshape(L, 8, 128).transpose(0, 2, 1) / G)
        csl = slice(1024 * r, 1024 * (r + 1))
        m["w1"] = _bf(np.stack([_shuf(W1_f[l][:, csl]) for l in range(L)]))
        m["ws1"] = np.ascontiguousarray(
            wsum1[:, csl].reshape(L, 8, 128).transpose(0, 2, 1))
        m["ng1"] = np.ascontiguousarray(
            (-bias1[:, csl]).reshape(L, 8, 128).transpose(0, 2, 1))
        m["w2"] = _bf(np.stack([_shuf(W2[l][csl, :]) for l in range(L)]))
        m["b28"] = np.ascontiguousarray(
            b2.reshape(L, 8, 128).transpose(0, 2, 1) / G)

        hsl = slice(256 * r, 256 * (r + 1))
        m["wq"] = _bf(_shuf(ca_wqkv[:, hsl]))
        m["wk"] = _bf(_shuf(ca_wqkv[:, np.arange(256) + H + 256 * r]))
        m["wv"] = _bf(_shuf(ca_wqkv[:, np.arange(256) + 2 * H + 256 * r]))
        m["bq"] = np.ascontiguousarray(
            ca_bqkv[hsl].reshape(2, 128).T)
        m["bk"] = np.ascontiguousarray(
            ca_bqkv[H + 256 * r:H + 256 * (r + 1)].reshape(2, 128).T)
        m["bv"] = np.ascontiguousarray(
            ca_bqkv[2 * H + 256 * r:2 * H + 256 * (r + 1)].reshape(2, 128).T)
        m["cawoT"] = _bf(np.concatenate([
            _shuf(np.ascontiguousarray(
                ca_wo[256 * r + 128 * h2:256 * r + 128 * (h2 + 1), :].T))
            .reshape(128, 8, 128) for h2 in range(2)], axis=2)
            .reshape(128, -1))
        in_maps.append(m)
    return in_maps, byte_seq


def run_device(inputs, trace=False):
    skip = (np.allclose(np.asarray(inputs["fn_g"]), 1.0)
            and np.allclose(np.asarray(inputs["fn_b"]), 0.0)
            and np.allclose(np.asarray(inputs["ca_ln_g"]), 1.0)
            and np.allclose(np.asarray(inputs["ca_ln_b"]), 0.0))
    key = ("nc", skip)
    if key not in _CACHE:
        _CACHE[key] = _trace(skip)
    nc = _CACHE[key]
    in_maps, byte_seq = _prep(inputs)
    res = run_bass_kernel_spmd(nc, in_maps, core_ids=list(range(NC)),
                               trace=trace)
    out = np.empty((B, S, V), np.float32)
    for b in range(B):
        ltab = res.results[b * G]["ltab"]             # [128, 512]
        tab = ltab.reshape(128, 2, 256).transpose(1, 0, 2).reshape(256, 256)
        out[b] = tab.T[byte_seq[b]]                   # [S, 256]
    return out, res


def kernel(**inputs) -> np.ndarray:
    out, _ = run_device(inputs, trace=False)
    return out
